# revision 37
# baseline (speedup 1.0000x reference)
"""Trainium2 Bass kernel for nn_End2EndGCN (gumbel-softmax GCN over 16384 tiny graphs).

Math (per graph n, derived from the reference):
  obs[658] -> x = relu(W0.T obs + b0)                       [64]
  d(i,j)   = (Wl[:,1]-Wl[:,0]).T x + (bl1-bl0) + gdiff      [35,35]   (softmax over 2 == sigmoid of diff)
  adj      = sigmoid(d)
  deg_i    = sum_j adj_ij ;  r_i = 1/(deg_i + 1e-6)
  w_j      = (1/35) sum_i adj_ij r_i ;  s = sum_j w_j
  emb      = sum_c obs_c * w_node(c) * KW[c,:] + s*bg       [128]     (KW = Wg rows by feature pos;
                                                                       s*bg rides a virtual all-ones obs row)

The gumbel noise gdiff is input-independent (fixed PRNG seed) -> computed once on
host with jax (exact threefry bits, one vmap over all N keys like the reference),
shipped to the device as fp16.

Layout: everything transposed, [feature, graph]; graphs sharded 8 ways (2048/core),
processed in 4 blocks of 512 per core. All matmul operands fp16 (PE runs fp16 at
1 cycle/row vs 4 for fp32), accumulation in fp32 PSUM. GCN normalization is done
with 0/1 selection-matrix matmuls on the PE; the gumbel add is a PE identity-matmul
accumulate. K<=64 matmuls (mm2 / rrep / wrep) are issued in row-group pairs
(tile_position via base partition 0/64) so two run concurrently in the PE array.
All constants ship as one packed [128, C] blob (single DMA); per-block obs/gd each
load with a single 3D-AP DMA from padded host arrays (HWDGE overhead is ~625ns per
dma_start, so DMA count matters).
"""

import numpy as np

import concourse.bass as bass
import concourse.bacc as bacc
import concourse.tile as tile
from concourse import mybir
from concourse import bass_utils

# ---------------------------------------------------------------- problem dims
T_, B_ = 32, 512
N_GRAPHS = T_ * B_            # 16384
OBS = 658
OBS_E = OBS + 1               # +1 virtual all-ones feature row carrying bg*s
HID = 64
NN = 35                       # nodes
NN2 = NN * NN                 # 1225
NODE_DIM = 128
SEED = 1
N_CORES = 8
PER_CORE = N_GRAPHS // N_CORES   # 2048
G = 512                          # graphs per block (PSUM fp32 free-dim limit)
N_BLOCKS = PER_CORE // G         # 4

OBS_PAD = 768                 # padded row counts for single-DMA 3D access patterns
IJ_PAD = 1280

PAIRING = True
GD_BF16 = True
BF16 = mybir.dt.bfloat16
F16 = mybir.dt.float16
F32 = mybir.dt.float32


def _chunks(total, size=128):
    out = []
    p = 0
    while p < total:
        out.append((p, min(size, total - p)))
        p += size
    return out


CH_OBS = _chunks(OBS_E)  # 6 chunks over 659
CH_IJ = _chunks(NN2)     # 10 chunks over 1225


# ---------------------------------------------------------------- host constants
def _feature_map():
    """col -> (node, k) mapping replicating reference.preprocess."""
    node = np.zeros(OBS, np.int64)
    kpos = np.zeros(OBS, np.int64)
    spans = []  # (node, obs_lo, obs_hi, k_lo)
    for i in range(5):                      # hands 5x25
        spans.append((i, 25 * i, 25 * i + 25, 0))
    spans.append((5, 125, 127, 0))          # hands missing card
    spans.append((6, 127, 167, 0))          # deck (40)
    spans.append((7, 167, 192, 0))          # fireworks (25)
    spans.append((8, 192, 200, 0))          # info tokens (8)
    spans.append((9, 200, 203, 0))          # life tokens (3)
    for i in range(5):                      # discards 5x10
        spans.append((10 + i, 203 + 10 * i, 213 + 10 * i, 0))
    la = [(0, 2), (2, 6), (6, 8), (8, 13), (13, 18), (18, 23), (23, 28), (28, 53), (53, 54), (54, 55)]
    for idx, (a, b) in enumerate(la):       # last action slices
        spans.append((15 + idx, 253 + a, 253 + b, 0))
    for i in range(10):                     # v0 nodes: 25 + 5 + 5
        spans.append((25 + i, 308 + 25 * i, 308 + 25 * i + 25, 0))
        spans.append((25 + i, 558 + 5 * i, 558 + 5 * i + 5, 25))
        spans.append((25 + i, 608 + 5 * i, 608 + 5 * i + 5, 30))
    for nd, lo, hi, k0 in spans:
        for c in range(lo, hi):
            node[c] = nd
            kpos[c] = k0 + (c - lo)
    return node, kpos


_NODE_OF_COL, _KPOS_OF_COL = _feature_map()

_CACHE = {}


def _gdiff_T():
    """[IJ_PAD, N] fp32: gumbel(1)-gumbel(0) transposed, zero-padded rows. Input-independent."""
    if "gdT" not in _CACHE:
        import jax

        cpu = jax.devices("cpu")[0]
        with jax.default_device(cpu):
            # Must mirror the reference formulation exactly: one vmap over all
            # N keys (vmap's threefry batching is not chunk-decomposable).
            rng = jax.random.PRNGKey(SEED)
            keys = jax.random.split(rng, N_GRAPHS)
            u = jax.vmap(lambda k: jax.random.uniform(k, (NN, NN, 2)))(keys)
            g = -jax.numpy.log(-jax.numpy.log(u))
            gd = np.asarray(g[..., 1] - g[..., 0], np.float32).reshape(N_GRAPHS, NN2)
        # u==0 gives g=-inf -> gdiff=+/-inf; sigmoid saturates exactly at +/-1000
        # already (matching the reference's softmax limit), and a finite value
        # avoids 0*inf=NaN in the identity-matmul gumbel add.
        gd = np.clip(gd, -1000.0, 1000.0)
        gdT = np.zeros((IJ_PAD, N_GRAPHS), np.float32)
        gdT[:NN2] = gd.T
        _CACHE["gdT"] = gdT
    return _CACHE["gdT"]


class _BlobLayout:
    """Column allocator for the packed [128, C] fp16 constant blob."""

    def __init__(self):
        self.cols = 0
        self.slots = {}

    def alloc(self, name, width):
        self.slots[name] = (self.cols, width)
        self.cols += width
        return self.slots[name]


def _blob_layout():
    L = _BlobLayout()
    for i, (p, w) in enumerate(CH_OBS):
        L.alloc(f"w0_{i}", HID)
    L.alloc("wld2", NN2)
    L.alloc("i128", 128)
    for i, (p, w) in enumerate(CH_IJ):
        L.alloc(f"e35_{i}", NN)
    for i, (p, w) in enumerate(CH_IJ):
        L.alloc(f"e35t_{i}", w)
    for i, (p, w) in enumerate(CH_IJ):
        L.alloc(f"f36_{i}", NN)
    for i, (p, w) in enumerate(CH_OBS):
        L.alloc(f"e658t_{i}", w)
    for i, (p, w) in enumerate(CH_OBS):
        L.alloc(f"kw_{i}", NODE_DIM)
    return L


_LAYOUT = _blob_layout()


def _build_blob(W0, Wld, KW):
    """Host-side packed constant blob [128, C] fp16.

    Selection matrices for the GCN normalization:
      e35_m  [w,35]: deg         degT += e35_m.T @ adjT_m
      e35t_m [35,w]: r broadcast rrepT_m = e35t_m.T @ rT   (dup at rows 64:99 for row-pair packing)
      f36_m  [w,35]: w-sum       wT += f36_m.T @ (adjT*rrepT)_m   (1/35 folded in)
      e658t_i[35,w]: w broadcast over obs cols (+ all-ones virtual row -> s)
      kw_i   [w,128]: fused preprocess-scatter @ Wg (+ bg row)
    """
    ii = np.arange(NN2) // NN
    jj = np.arange(NN2) % NN
    E35 = np.zeros((NN2, NN), np.float16)
    E35[np.arange(NN2), ii] = 1
    F36 = np.zeros((NN2, NN), np.float16)
    F36[np.arange(NN2), jj] = np.float16(1.0 / NN)
    E658T = np.zeros((NN, OBS_E), np.float16)
    E658T[_NODE_OF_COL, np.arange(OBS)] = 1
    E658T[:, OBS] = 1

    blob = np.zeros((128, _LAYOUT.cols), np.float16)

    def put(name, rows, data):
        c0, w = _LAYOUT.slots[name]
        blob[rows, c0:c0 + data.shape[1]] = data

    for i, (p, w) in enumerate(CH_OBS):
        w658 = min(w, OBS - p)
        put(f"w0_{i}", slice(0, w658), W0[p:p + w658, :])
    c0, _ = _LAYOUT.slots["wld2"]
    blob[0:HID, c0:c0 + NN2] = Wld
    blob[64:64 + HID, c0:c0 + NN2] = Wld
    c0, _ = _LAYOUT.slots["i128"]
    blob[:, c0:c0 + 128] = np.eye(128, dtype=np.float16)
    for i, (p, w) in enumerate(CH_IJ):
        put(f"e35_{i}", slice(0, w), E35[p:p + w, :])
        put(f"f36_{i}", slice(0, w), F36[p:p + w, :])
        put(f"e35t_{i}", slice(0, NN), E35[p:p + w, :].T)
        put(f"e35t_{i}", slice(64, 64 + NN), E35[p:p + w, :].T)
    for i, (p, w) in enumerate(CH_OBS):
        put(f"e658t_{i}", slice(0, NN), E658T[:, p:p + w])
        put(f"e658t_{i}", slice(64, 64 + NN), E658T[:, p:p + w])
        put(f"kw_{i}", slice(0, w), KW[p:p + w, :])
    return blob


# ---------------------------------------------------------------- device program
def _build_program(reps=1):
    nc = bacc.Bacc("TRN2", target_bir_lowering=False, debug=False, num_devices=N_CORES)

    obsT = nc.dram_tensor("obsT", [OBS_PAD, PER_CORE], F16, kind="ExternalInput").ap()
    gdT = nc.dram_tensor("gdT", [IJ_PAD, PER_CORE], BF16 if GD_BF16 else F16, kind="ExternalInput").ap()
    blob_d = nc.dram_tensor("blob", [128, _LAYOUT.cols], F16, kind="ExternalInput").ap()
    b0 = nc.dram_tensor("b0", [HID, 1], F32, kind="ExternalInput").ap()
    embT = nc.dram_tensor("embT", [NODE_DIM, PER_CORE], F16, kind="ExternalOutput").ap()

    obsT3 = obsT.rearrange("(m p) g -> p m g", p=128)   # [128, 6, PER_CORE]
    gdT3 = gdT.rearrange("(m p) g -> p m g", p=128)     # [128, 10, PER_CORE]

    AF = mybir.ActivationFunctionType
    NCH_O = len(CH_OBS)
    NCH_IJ = len(CH_IJ)

    with tile.TileContext(nc) as tc:
        with (
            tc.tile_pool(name="singles", bufs=1) as singles,
            tc.tile_pool(name="obs_p", bufs=3) as obs_p,
            tc.tile_pool(name="gd_p", bufs=3) as gd_p,
            tc.tile_pool(name="adj_p", bufs=3) as adj_p,
            tc.tile_pool(name="sm_p", bufs=4) as sm_p,
            tc.tile_pool(name="radj_p", bufs=5) as radj_p,
            tc.tile_pool(name="wnf_p", bufs=5) as wnf_p,
            tc.tile_pool(name="out_p", bufs=3) as out_p,
            tc.tile_pool(name="ps_d", bufs=3, space="PSUM") as ps_d,
            tc.tile_pool(name="ps_deg", bufs=2, space="PSUM") as ps_deg,
            tc.tile_pool(name="ps_r", bufs=2, space="PSUM") as ps_r,
            tc.tile_pool(name="ps_w", bufs=1, space="PSUM") as ps_w,
            tc.tile_pool(name="ps_e", bufs=1, space="PSUM") as ps_e,
        ):
            blob = singles.tile([128, _LAYOUT.cols], F16, tag="blob", name="blob")
            _chead = _LAYOUT.slots["e35_0"][0]   # w0+wld2+i128 prefix needed first
            nc.sync.dma_start(out=blob[:, 0:_chead], in_=blob_d[:, 0:_chead])
            nc.sync.dma_start(out=blob[:, _chead:], in_=blob_d[:, _chead:])
            b0_t = singles.tile([HID, 1], F32, tag="b0", name="b0")
            nc.sync.dma_start(out=b0_t, in_=b0)
            eps_t = singles.tile([NN, 1], F32, tag="eps", name="eps")
            nc.vector.memset(eps_t, 1e-6)

            def bl(name, r0, r1):
                c0, w = _LAYOUT.slots[name]
                return blob[r0:r1, c0:c0 + w]

            cwld, _ = _LAYOUT.slots["wld2"]

            # -------- phase A (adjacency) of block `blk`: list of op-groups
            def A_groups(blk, st):
                g0 = blk * G
                gs = []

                def load():
                    st["obs"] = obs_p.tile([128, NCH_O, G], F16, tag="obs", name="obs_t")
                    nc.sync.dma_start(out=st["obs"], in_=obsT3[:, :, g0:g0 + G])
                    st["gd"] = gd_p.tile([128, NCH_IJ, G], BF16 if GD_BF16 else F16, tag="gd", name="gd_t")
                    nc.sync.dma_start(out=st["gd"], in_=gdT3[:, :, g0:g0 + G])
                gs.append(load)

                def mm1():
                    # x = relu(W0.T obs + b0) [64, G], duplicated at rows 64:128
                    xps = ps_d.tile([128, G], F32, tag="dps", name="xps")
                    for i, (p, w) in enumerate(CH_OBS):
                        w658 = min(w, OBS - p)
                        nc.tensor.matmul(xps[0:HID, :], bl(f"w0_{i}", 0, w658), st["obs"][0:w658, i, :],
                                         start=(i == 0), stop=(i == NCH_O - 1))
                    x2 = sm_p.tile([128, G], F16, tag="x2", name="x2")
                    nc.scalar.activation(x2[0:HID, :], xps[0:HID, :], AF.Relu, bias=b0_t)
                    nc.gpsimd.tensor_copy(x2[64:64 + HID, :], x2[0:HID, :])
                    st["x2"] = x2
                gs.append(mm1)

                st["adj"] = []
                st["degq"] = []

                def adj_pair(pr):
                    def run():
                        m0, m1 = 2 * pr, 2 * pr + 1
                        if pr == 0:
                            st["degps"] = ps_deg.tile([NN, G], F32, tag="degps", name="degps")
                        x2 = st["x2"]
                        dtiles = []
                        for m, rlo in ((m0, 0), (m1, 64 if PAIRING else 0)):
                            p, w = CH_IJ[m]
                            dps = ps_d.tile([128, G], F32, tag="dps", name="dps")
                            nc.tensor.matmul(dps[0:w, :],
                                             blob[rlo:rlo + HID, cwld + p:cwld + p + w],
                                             x2[rlo:rlo + HID, :], start=True, stop=False,
                                             skip_group_check=True)
                            dtiles.append(dps)
                        for m, dps in zip((m0, m1), dtiles):
                            p, w = CH_IJ[m]
                            nc.tensor.matmul(dps[0:w, :], bl("i128", 0, w)[:, 0:w],
                                             st["gd"][0:w, m, :],
                                             start=False, stop=True, skip_group_check=True)
                            adj = adj_p.tile([128, G], F16, tag=f"adj_{m}", name=f"adj_{m}")
                            nc.scalar.activation(adj[0:w, :], dps[0:w, :], AF.Sigmoid)
                            st["adj"].append(adj)
                            st["degq"].append(m)
                        # deg matmuls for the PREVIOUS pair (its sigmoids are done ->
                        # PE does not stall on ACT)
                        while len(st["degq"]) > 2:
                            m = st["degq"].pop(0)
                            p, w = CH_IJ[m]
                            nc.tensor.matmul(st["degps"], bl(f"e35_{m}", 0, w),
                                             st["adj"][m][0:w, :],
                                             start=(m == 0), stop=(m == NCH_IJ - 1),
                                             skip_group_check=True)
                    return run
                for pr in range(NCH_IJ // 2):
                    gs.append(adj_pair(pr))

                def head():
                    while st["degq"]:
                        m = st["degq"].pop(0)
                        p, w = CH_IJ[m]
                        nc.tensor.matmul(st["degps"], bl(f"e35_{m}", 0, w),
                                         st["adj"][m][0:w, :],
                                         start=(m == 0), stop=(m == NCH_IJ - 1),
                                         skip_group_check=True)
                    # r = 1/(deg + 1e-6) fp16, duplicated at rows 64:99
                    deg1 = sm_p.tile([NN, G], F32, tag="deg1", name="deg1")
                    nc.scalar.activation(deg1, st["degps"], AF.Identity, bias=eps_t)
                    r2 = sm_p.tile([64 + NN, G], F16, tag="r2", name="r2")
                    with nc.allow_low_precision("fp16 r is within tolerance"):
                        nc.vector.reciprocal(r2[0:NN, :], deg1)
                        nc.vector.reciprocal(r2[64:64 + NN, :], deg1)
                    st["r2"] = r2
                gs.append(head)
                return gs

            # -------- phase B (normalize + readout) of block `blk`
            def B_groups(blk, st):
                g0 = blk * G
                gs = []

                st["wq"] = []

                def rw_pair(pr):
                    def run():
                        m0, m1 = 2 * pr, 2 * pr + 1
                        if pr == 0:
                            st["wps"] = ps_w.tile([NN, G], F32, tag="wps", name="wps")
                        r2 = st["r2"]
                        rtiles = []
                        for m, rlo in ((m0, 0), (m1, 64 if PAIRING else 0)):
                            p, w = CH_IJ[m]
                            rrps = ps_r.tile([128, G], F32, tag="rrps", name="rrps")
                            nc.tensor.matmul(rrps[0:w, :], bl(f"e35t_{m}", rlo, rlo + NN),
                                             r2[rlo:rlo + NN, :], start=True, stop=True,
                                             skip_group_check=True)
                            rtiles.append(rrps)
                        for m, rrps in zip((m0, m1), rtiles):
                            p, w = CH_IJ[m]
                            radj = radj_p.tile([128, G], F16, tag="radj", name="radj")
                            nc.vector.tensor_mul(radj[0:w, :], st["adj"][m][0:w, :], rrps[0:w, :])
                            st["wq"].append((m, radj))
                        while len(st["wq"]) > 2:
                            m, radj = st["wq"].pop(0)
                            p, w = CH_IJ[m]
                            nc.tensor.matmul(st["wps"], bl(f"f36_{m}", 0, w), radj[0:w, :],
                                             start=(m == 0), stop=(m == NCH_IJ - 1),
                                             skip_group_check=True)
                    return run
                for pr in range(NCH_IJ // 2):
                    gs.append(rw_pair(pr))

                def whead():
                    while st["wq"]:
                        m, radj = st["wq"].pop(0)
                        p, w = CH_IJ[m]
                        nc.tensor.matmul(st["wps"], bl(f"f36_{m}", 0, w), radj[0:w, :],
                                         start=(m == 0), stop=(m == NCH_IJ - 1),
                                         skip_group_check=True)
                    w2 = sm_p.tile([64 + NN, G], F16, tag="w2", name="w2")
                    nc.scalar.copy(w2[0:NN, :], st["wps"])
                    nc.scalar.copy(w2[64:64 + NN, :], st["wps"])
                    st["w2"] = w2
                gs.append(whead)

                st["eq"] = []

                def ro_pair(pr):
                    def run():
                        i0, i1 = 2 * pr, 2 * pr + 1
                        if pr == 0:
                            st["embps"] = ps_e.tile([NODE_DIM, G], F32, tag="embps", name="embps")
                        w2 = st["w2"]
                        wtiles = []
                        for i, rlo in ((i0, 0), (i1, 64 if PAIRING else 0)):
                            p, w = CH_OBS[i]
                            wrps = ps_r.tile([128, G], F32, tag="rrps", name="wrps")
                            nc.tensor.matmul(wrps[0:w, :], bl(f"e658t_{i}", rlo, rlo + NN),
                                             w2[rlo:rlo + NN, :], start=True, stop=True,
                                             skip_group_check=True)
                            wtiles.append(wrps)
                        for i, wrps in zip((i0, i1), wtiles):
                            p, w = CH_OBS[i]
                            wnf = wnf_p.tile([128, G], F16, tag="wnf", name="wnf")
                            nc.vector.tensor_mul(wnf[0:w, :], st["obs"][0:w, i, :], wrps[0:w, :])
                            st["eq"].append((i, wnf))
                        while len(st["eq"]) > 2:
                            i, wnf = st["eq"].pop(0)
                            p, w = CH_OBS[i]
                            nc.tensor.matmul(st["embps"], bl(f"kw_{i}", 0, w), wnf[0:w, :],
                                             start=(i == 0), stop=(i == NCH_O - 1),
                                             skip_group_check=True)
                    return run
                for pr in range(NCH_O // 2):
                    gs.append(ro_pair(pr))

                def out():
                    while st["eq"]:
                        i, wnf = st["eq"].pop(0)
                        p, w = CH_OBS[i]
                        nc.tensor.matmul(st["embps"], bl(f"kw_{i}", 0, w), wnf[0:w, :],
                                         start=(i == 0), stop=(i == NCH_O - 1),
                                         skip_group_check=True)
                    emb = out_p.tile([NODE_DIM, G], F16, tag="emb", name="emb")
                    nc.scalar.copy(emb, st["embps"])
                    nc.sync.dma_start(out=embT[:, g0:g0 + G], in_=emb)
                gs.append(out)
                return gs

            def interleave(a, b):
                # proportional round-robin, starting with a
                out, ia, ib, na, nb = [], 0, 0, len(a), len(b)
                while ia < na or ib < nb:
                    if ib < nb and (ia >= na or ib * na <= ia * nb):
                        out.append(b[ib]); ib += 1
                    else:
                        out.append(a[ia]); ia += 1
                return out

            # software pipeline: phase A of block k runs interleaved with phase B
            # of block k-1 so PE/DVE/ACT overlap across blocks.
            sts = {}
            aq = []   # blocks whose A phase is emitted, B pending
            for rep in range(reps):
                for blk in range(N_BLOCKS):
                    key = (rep, blk)
                    sts[key] = {}
                    ga = A_groups(blk, sts[key])
                    if rep == 0 and blk == 0:
                        for fn in ga:
                            fn()
                    elif rep == 0 and blk == 1:
                        # warm-up: overlap A1 with nothing pending yet beyond A0's tail
                        for fn in ga:
                            fn()
                        aq.append((0, (rep, 0)))
                        continue
                    else:
                        bkey = aq.pop(0)
                        gb = B_groups(bkey[0], sts[bkey[1]])
                        for fn in interleave(ga, gb):
                            fn()
                    aq.append((blk, key))
            while aq:
                bblk, bkey = aq.pop(0)
                for fn in B_groups(bblk, sts[bkey]):
                    fn()

    nc.finalize()
    return nc


def _get_program(reps=1):
    key = ("prog", reps, PAIRING, GD_BF16)
    if key not in _CACHE:
        _CACHE[key] = _build_program(reps)
    return _CACHE[key]


# ---------------------------------------------------------------- entry point
def _prep_inputs(observations, W0, b0, Wl, bl, Wg, bg):
    obs = np.asarray(observations, np.float32).reshape(N_GRAPHS, OBS)
    obsT16 = np.zeros((OBS_PAD, N_GRAPHS), np.float16)
    obsT16[:OBS] = obs.T.astype(np.float16)
    obsT16[OBS] = 1.0                                            # virtual ones row

    Wl = np.asarray(Wl, np.float32)
    bl = np.asarray(bl, np.float32)
    Wld16 = (Wl[:, 1::2] - Wl[:, 0::2]).astype(np.float16)       # [64,1225]
    bld = np.zeros((IJ_PAD, 1), np.float32)
    bld[:NN2, 0] = bl[1::2] - bl[0::2]

    import ml_dtypes
    gdT16 = (_gdiff_T() + bld).astype(ml_dtypes.bfloat16 if GD_BF16 else np.float16)

    Wg = np.asarray(Wg, np.float32)
    KW = np.empty((OBS_E, NODE_DIM), np.float32)
    KW[:OBS] = Wg[_KPOS_OF_COL, :]
    KW[OBS] = np.asarray(bg, np.float32)

    blob = _build_blob(np.asarray(W0, np.float32).astype(np.float16),
                       Wld16, KW.astype(np.float16))

    common = {"blob": blob, "b0": np.asarray(b0, np.float32).reshape(HID, 1)}
    in_maps = []
    for c in range(N_CORES):
        s = slice(c * PER_CORE, (c + 1) * PER_CORE)
        m = dict(common)
        m["obsT"] = np.ascontiguousarray(obsT16[:, s])
        m["gdT"] = np.ascontiguousarray(gdT16[:, s])
        in_maps.append(m)
    return in_maps


def _run(inputs, reps=1):
    nc = _get_program(reps)
    in_maps = _prep_inputs(**inputs)
    res = bass_utils.run_bass_kernel_spmd(nc, in_maps, core_ids=list(range(N_CORES)))
    outs = [res.results[c]["embT"] for c in range(N_CORES)]          # each [128, 2048] fp16
    embT = np.concatenate(outs, axis=1).astype(np.float32)           # [128, N]
    return np.ascontiguousarray(embT.T).reshape(T_, B_, NODE_DIM)


def kernel(**inputs):
    return _run(inputs, reps=1)


# revision 40
# speedup vs baseline: 1.0222x; 1.0222x over previous
"""Trainium2 Bass kernel for nn_End2EndGCN (gumbel-softmax GCN over 16384 tiny graphs).

Math (per graph n, derived from the reference):
  obs[658] -> x = relu(W0.T obs + b0)                       [64]
  d(i,j)   = (Wl[:,1]-Wl[:,0]).T x + (bl1-bl0) + gdiff      [35,35]   (softmax over 2 == sigmoid of diff)
  adj      = sigmoid(d)
  deg_i    = sum_j adj_ij ;  r_i = 1/(deg_i + 1e-6)
  w_j      = (1/35) sum_i adj_ij r_i ;  s = sum_j w_j
  emb      = sum_c obs_c * w_node(c) * KW[c,:] + s*bg       [128]     (KW = Wg rows by feature pos;
                                                                       s*bg rides a virtual all-ones obs row)

The gumbel noise gdiff is input-independent (fixed PRNG seed) -> computed once on
host with jax (exact threefry bits, one vmap over all N keys like the reference),
shipped to the device as fp16.

Layout: everything transposed, [feature, graph]; graphs sharded 8 ways (2048/core),
processed in 4 blocks of 512 per core. All matmul operands fp16 (PE runs fp16 at
1 cycle/row vs 4 for fp32), accumulation in fp32 PSUM. GCN normalization is done
with 0/1 selection-matrix matmuls on the PE; the gumbel add is a PE identity-matmul
accumulate. K<=64 matmuls (mm2 / rrep / wrep) are issued in row-group pairs
(tile_position via base partition 0/64) so two run concurrently in the PE array.
All constants ship as one packed [128, C] blob (single DMA); per-block obs/gd each
load with a single 3D-AP DMA from padded host arrays (HWDGE overhead is ~625ns per
dma_start, so DMA count matters).
"""

import numpy as np

import concourse.bass as bass
import concourse.bacc as bacc
import concourse.tile as tile
from concourse import mybir
from concourse import bass_utils

# ---------------------------------------------------------------- problem dims
T_, B_ = 32, 512
N_GRAPHS = T_ * B_            # 16384
OBS = 658
OBS_E = OBS + 1               # +1 virtual all-ones feature row carrying bg*s
HID = 64
NN = 35                       # nodes
NN2 = NN * NN                 # 1225
NODE_DIM = 128
SEED = 1
N_CORES = 8
PER_CORE = N_GRAPHS // N_CORES   # 2048
G = 512                          # graphs per block (PSUM fp32 free-dim limit)
N_BLOCKS = PER_CORE // G         # 4

OBS_PAD = 768                 # padded row counts for single-DMA 3D access patterns
IJ_PAD = 1280

PAIRING = True
GD_BF16 = True
BF16 = mybir.dt.bfloat16
F16 = mybir.dt.float16
F32 = mybir.dt.float32


def _chunks(total, size=128):
    out = []
    p = 0
    while p < total:
        out.append((p, min(size, total - p)))
        p += size
    return out


CH_OBS = _chunks(OBS_E)  # 6 chunks over 659
CH_IJ = _chunks(NN2)     # 10 chunks over 1225


# ---------------------------------------------------------------- host constants
def _feature_map():
    """col -> (node, k) mapping replicating reference.preprocess."""
    node = np.zeros(OBS, np.int64)
    kpos = np.zeros(OBS, np.int64)
    spans = []  # (node, obs_lo, obs_hi, k_lo)
    for i in range(5):                      # hands 5x25
        spans.append((i, 25 * i, 25 * i + 25, 0))
    spans.append((5, 125, 127, 0))          # hands missing card
    spans.append((6, 127, 167, 0))          # deck (40)
    spans.append((7, 167, 192, 0))          # fireworks (25)
    spans.append((8, 192, 200, 0))          # info tokens (8)
    spans.append((9, 200, 203, 0))          # life tokens (3)
    for i in range(5):                      # discards 5x10
        spans.append((10 + i, 203 + 10 * i, 213 + 10 * i, 0))
    la = [(0, 2), (2, 6), (6, 8), (8, 13), (13, 18), (18, 23), (23, 28), (28, 53), (53, 54), (54, 55)]
    for idx, (a, b) in enumerate(la):       # last action slices
        spans.append((15 + idx, 253 + a, 253 + b, 0))
    for i in range(10):                     # v0 nodes: 25 + 5 + 5
        spans.append((25 + i, 308 + 25 * i, 308 + 25 * i + 25, 0))
        spans.append((25 + i, 558 + 5 * i, 558 + 5 * i + 5, 25))
        spans.append((25 + i, 608 + 5 * i, 608 + 5 * i + 5, 30))
    for nd, lo, hi, k0 in spans:
        for c in range(lo, hi):
            node[c] = nd
            kpos[c] = k0 + (c - lo)
    return node, kpos


_NODE_OF_COL, _KPOS_OF_COL = _feature_map()

_CACHE = {}


def _gdiff_T():
    """[IJ_PAD, N] fp32: gumbel(1)-gumbel(0) transposed, zero-padded rows. Input-independent."""
    if "gdT" not in _CACHE:
        import jax

        cpu = jax.devices("cpu")[0]
        with jax.default_device(cpu):
            # Must mirror the reference formulation exactly: one vmap over all
            # N keys (vmap's threefry batching is not chunk-decomposable).
            rng = jax.random.PRNGKey(SEED)
            keys = jax.random.split(rng, N_GRAPHS)
            u = jax.vmap(lambda k: jax.random.uniform(k, (NN, NN, 2)))(keys)
            g = -jax.numpy.log(-jax.numpy.log(u))
            gd = np.asarray(g[..., 1] - g[..., 0], np.float32).reshape(N_GRAPHS, NN2)
        # u==0 gives g=-inf -> gdiff=+/-inf; sigmoid saturates exactly at +/-1000
        # already (matching the reference's softmax limit), and a finite value
        # avoids 0*inf=NaN in the identity-matmul gumbel add.
        gd = np.clip(gd, -1000.0, 1000.0)
        gdT = np.zeros((IJ_PAD, N_GRAPHS), np.float32)
        gdT[:NN2] = gd.T
        _CACHE["gdT"] = gdT
    return _CACHE["gdT"]


class _BlobLayout:
    """Column allocator for the packed [128, C] fp16 constant blob."""

    def __init__(self):
        self.cols = 0
        self.slots = {}

    def alloc(self, name, width):
        self.slots[name] = (self.cols, width)
        self.cols += width
        return self.slots[name]


def _blob_layout():
    L = _BlobLayout()
    for i, (p, w) in enumerate(CH_OBS):
        L.alloc(f"w0_{i}", HID)
    L.alloc("wld2", NN2)
    L.alloc("i128", 128)
    for i, (p, w) in enumerate(CH_IJ):
        L.alloc(f"e35_{i}", NN)
    for i, (p, w) in enumerate(CH_IJ):
        L.alloc(f"e35t_{i}", w)
    for i, (p, w) in enumerate(CH_IJ):
        L.alloc(f"f36_{i}", NN)
    for i, (p, w) in enumerate(CH_OBS):
        L.alloc(f"e658t_{i}", w)
    for i, (p, w) in enumerate(CH_OBS):
        L.alloc(f"kw_{i}", NODE_DIM)
    return L


_LAYOUT = _blob_layout()


def _build_blob(W0, Wld, KW):
    """Host-side packed constant blob [128, C] fp16.

    Selection matrices for the GCN normalization:
      e35_m  [w,35]: deg         degT += e35_m.T @ adjT_m
      e35t_m [35,w]: r broadcast rrepT_m = e35t_m.T @ rT   (dup at rows 64:99 for row-pair packing)
      f36_m  [w,35]: w-sum       wT += f36_m.T @ (adjT*rrepT)_m   (1/35 folded in)
      e658t_i[35,w]: w broadcast over obs cols (+ all-ones virtual row -> s)
      kw_i   [w,128]: fused preprocess-scatter @ Wg (+ bg row)
    """
    ii = np.arange(NN2) // NN
    jj = np.arange(NN2) % NN
    E35 = np.zeros((NN2, NN), np.float16)
    E35[np.arange(NN2), ii] = 1
    F36 = np.zeros((NN2, NN), np.float16)
    F36[np.arange(NN2), jj] = np.float16(1.0 / NN)
    E658T = np.zeros((NN, OBS_E), np.float16)
    E658T[_NODE_OF_COL, np.arange(OBS)] = 1
    E658T[:, OBS] = 1

    blob = np.zeros((128, _LAYOUT.cols), np.float16)

    def put(name, rows, data):
        c0, w = _LAYOUT.slots[name]
        blob[rows, c0:c0 + data.shape[1]] = data

    for i, (p, w) in enumerate(CH_OBS):
        w658 = min(w, OBS - p)
        put(f"w0_{i}", slice(0, w658), W0[p:p + w658, :])
    c0, _ = _LAYOUT.slots["wld2"]
    blob[0:HID, c0:c0 + NN2] = Wld
    blob[64:64 + HID, c0:c0 + NN2] = Wld
    c0, _ = _LAYOUT.slots["i128"]
    blob[:, c0:c0 + 128] = np.eye(128, dtype=np.float16)
    for i, (p, w) in enumerate(CH_IJ):
        put(f"e35_{i}", slice(0, w), E35[p:p + w, :])
        put(f"f36_{i}", slice(0, w), F36[p:p + w, :])
        put(f"e35t_{i}", slice(0, NN), E35[p:p + w, :].T)
        put(f"e35t_{i}", slice(64, 64 + NN), E35[p:p + w, :].T)
    for i, (p, w) in enumerate(CH_OBS):
        put(f"e658t_{i}", slice(0, NN), E658T[:, p:p + w])
        put(f"e658t_{i}", slice(64, 64 + NN), E658T[:, p:p + w])
        put(f"kw_{i}", slice(0, w), KW[p:p + w, :])
    return blob


# ---------------------------------------------------------------- device program
def _build_program(reps=1):
    nc = bacc.Bacc("TRN2", target_bir_lowering=False, debug=False, num_devices=N_CORES)

    obsT = nc.dram_tensor("obsT", [OBS_PAD, PER_CORE], F16, kind="ExternalInput").ap()
    gdT = nc.dram_tensor("gdT", [IJ_PAD, PER_CORE], BF16 if GD_BF16 else F16, kind="ExternalInput").ap()
    blob_d = nc.dram_tensor("blob", [128, _LAYOUT.cols], F16, kind="ExternalInput").ap()
    b0 = nc.dram_tensor("b0", [HID, 1], F32, kind="ExternalInput").ap()
    embT = nc.dram_tensor("embT", [NODE_DIM, PER_CORE], F16, kind="ExternalOutput").ap()

    obsT3 = obsT.rearrange("(m p) g -> p m g", p=128)   # [128, 6, PER_CORE]
    gdT3 = gdT.rearrange("(m p) g -> p m g", p=128)     # [128, 10, PER_CORE]

    AF = mybir.ActivationFunctionType
    NCH_O = len(CH_OBS)
    NCH_IJ = len(CH_IJ)

    with tile.TileContext(nc) as tc:
        with (
            tc.tile_pool(name="singles", bufs=1) as singles,
            tc.tile_pool(name="obs_p", bufs=3) as obs_p,
            tc.tile_pool(name="gd_p", bufs=3) as gd_p,
            tc.tile_pool(name="adj_p", bufs=3) as adj_p,
            tc.tile_pool(name="sm_p", bufs=4) as sm_p,
            tc.tile_pool(name="radj_p", bufs=5) as radj_p,
            tc.tile_pool(name="wnf_p", bufs=5) as wnf_p,
            tc.tile_pool(name="out_p", bufs=3) as out_p,
            tc.tile_pool(name="ps_d", bufs=3, space="PSUM") as ps_d,
            tc.tile_pool(name="ps_deg", bufs=2, space="PSUM") as ps_deg,
            tc.tile_pool(name="ps_r", bufs=2, space="PSUM") as ps_r,
            tc.tile_pool(name="ps_w", bufs=1, space="PSUM") as ps_w,
            tc.tile_pool(name="ps_e", bufs=1, space="PSUM") as ps_e,
        ):
            blob = singles.tile([128, _LAYOUT.cols], F16, tag="blob", name="blob")
            _chead = _LAYOUT.slots["e35_0"][0]   # w0+wld2+i128 prefix needed first
            nc.sync.dma_start(out=blob[:, 0:_chead], in_=blob_d[:, 0:_chead])
            nc.sync.dma_start(out=blob[:, _chead:], in_=blob_d[:, _chead:])
            b0_t = singles.tile([HID, 1], F32, tag="b0", name="b0")
            nc.sync.dma_start(out=b0_t, in_=b0)
            eps_t = singles.tile([NN, 1], F32, tag="eps", name="eps")
            nc.vector.memset(eps_t, 1e-6)

            def bl(name, r0, r1):
                c0, w = _LAYOUT.slots[name]
                return blob[r0:r1, c0:c0 + w]

            cwld, _ = _LAYOUT.slots["wld2"]

            # -------- phase A (adjacency) of block `blk`: list of op-groups
            def A_groups(blk, st):
                g0 = blk * G
                gs = []

                def load():
                    st["obs"] = obs_p.tile([128, NCH_O, G], F16, tag="obs", name="obs_t")
                    nc.sync.dma_start(out=st["obs"], in_=obsT3[:, :, g0:g0 + G])
                    st["gd"] = gd_p.tile([128, NCH_IJ, G], BF16 if GD_BF16 else F16, tag="gd", name="gd_t")
                    nc.sync.dma_start(out=st["gd"], in_=gdT3[:, :, g0:g0 + G])
                gs.append(load)

                def mm1():
                    # x = relu(W0.T obs + b0) [64, G], duplicated at rows 64:128
                    xps = ps_d.tile([128, G], F32, tag="dps", name="xps")
                    for i, (p, w) in enumerate(CH_OBS):
                        w658 = min(w, OBS - p)
                        nc.tensor.matmul(xps[0:HID, :], bl(f"w0_{i}", 0, w658), st["obs"][0:w658, i, :],
                                         start=(i == 0), stop=(i == NCH_O - 1))
                    x2 = sm_p.tile([128, G], F16, tag="x2", name="x2")
                    nc.scalar.activation(x2[0:HID, :], xps[0:HID, :], AF.Relu, bias=b0_t)
                    nc.gpsimd.tensor_copy(x2[64:64 + HID, :], x2[0:HID, :])
                    st["x2"] = x2
                gs.append(mm1)

                st["adj"] = []
                st["degq"] = []

                def adj_pair(pr):
                    def run():
                        m0, m1 = 2 * pr, 2 * pr + 1
                        if pr == 0:
                            st["degps"] = ps_deg.tile([NN, G], F32, tag="degps", name="degps")
                        x2 = st["x2"]
                        dtiles = []
                        for m, rlo in ((m0, 0), (m1, 64 if PAIRING else 0)):
                            p, w = CH_IJ[m]
                            dps = ps_d.tile([128, G], F32, tag="dps", name="dps")
                            nc.tensor.matmul(dps[0:w, :],
                                             blob[rlo:rlo + HID, cwld + p:cwld + p + w],
                                             x2[rlo:rlo + HID, :], start=True, stop=False,
                                             skip_group_check=True)
                            dtiles.append(dps)
                        for m, dps in zip((m0, m1), dtiles):
                            p, w = CH_IJ[m]
                            nc.tensor.matmul(dps[0:w, :], bl("i128", 0, w)[:, 0:w],
                                             st["gd"][0:w, m, :],
                                             start=False, stop=True, skip_group_check=True)
                            adj = adj_p.tile([128, G], F16, tag=f"adj_{m}", name=f"adj_{m}")
                            nc.scalar.activation(adj[0:w, :], dps[0:w, :], AF.Sigmoid)
                            st["adj"].append(adj)
                            st["degq"].append(m)
                        # deg matmuls for the PREVIOUS pair (its sigmoids are done ->
                        # PE does not stall on ACT)
                        while len(st["degq"]) > 2:
                            m = st["degq"].pop(0)
                            p, w = CH_IJ[m]
                            nc.tensor.matmul(st["degps"], bl(f"e35_{m}", 0, w),
                                             st["adj"][m][0:w, :],
                                             start=(m == 0), stop=(m == NCH_IJ - 1),
                                             skip_group_check=True)
                    return run
                for pr in range(NCH_IJ // 2):
                    gs.append(adj_pair(pr))

                def head():
                    while st["degq"]:
                        m = st["degq"].pop(0)
                        p, w = CH_IJ[m]
                        nc.tensor.matmul(st["degps"], bl(f"e35_{m}", 0, w),
                                         st["adj"][m][0:w, :],
                                         start=(m == 0), stop=(m == NCH_IJ - 1),
                                         skip_group_check=True)
                    # r = 1/deg fp16 straight from PSUM, duplicated at rows 64:99.
                    # (reference adds 1e-6; deg >= ~14 for this input distribution,
                    # so the epsilon shifts r by < 1e-7 relative -- far below the
                    # fp16 quantization already present.)
                    r2 = sm_p.tile([64 + NN, G], F16, tag="r2", name="r2")
                    with nc.allow_low_precision("fp16 r is within tolerance"):
                        nc.vector.reciprocal(r2[0:NN, :], st["degps"])
                    # fp16 SBUF copy runs in DVE 4x mode (194ns vs 654ns recip)
                    nc.vector.tensor_copy(r2[64:64 + NN, :], r2[0:NN, :])
                    st["r2"] = r2
                gs.append(head)
                return gs

            # -------- phase B (normalize + readout) of block `blk`
            def B_groups(blk, st):
                g0 = blk * G
                gs = []

                st["wq"] = []

                def rw_pair(pr):
                    def run():
                        m0, m1 = 2 * pr, 2 * pr + 1
                        if pr == 0:
                            st["wps"] = ps_w.tile([NN, G], F32, tag="wps", name="wps")
                        r2 = st["r2"]
                        rtiles = []
                        for m, rlo in ((m0, 0), (m1, 64 if PAIRING else 0)):
                            p, w = CH_IJ[m]
                            rrps = ps_r.tile([128, G], F32, tag="rrps", name="rrps")
                            nc.tensor.matmul(rrps[0:w, :], bl(f"e35t_{m}", rlo, rlo + NN),
                                             r2[rlo:rlo + NN, :], start=True, stop=True,
                                             skip_group_check=True)
                            rtiles.append(rrps)
                        for m, rrps in zip((m0, m1), rtiles):
                            p, w = CH_IJ[m]
                            radj = radj_p.tile([128, G], F16, tag="radj", name="radj")
                            nc.vector.tensor_mul(radj[0:w, :], st["adj"][m][0:w, :], rrps[0:w, :])
                            st["wq"].append((m, radj))
                        while len(st["wq"]) > 2:
                            m, radj = st["wq"].pop(0)
                            p, w = CH_IJ[m]
                            nc.tensor.matmul(st["wps"], bl(f"f36_{m}", 0, w), radj[0:w, :],
                                             start=(m == 0), stop=(m == NCH_IJ - 1),
                                             skip_group_check=True)
                    return run
                for pr in range(NCH_IJ // 2):
                    gs.append(rw_pair(pr))

                def whead():
                    while st["wq"]:
                        m, radj = st["wq"].pop(0)
                        p, w = CH_IJ[m]
                        nc.tensor.matmul(st["wps"], bl(f"f36_{m}", 0, w), radj[0:w, :],
                                         start=(m == 0), stop=(m == NCH_IJ - 1),
                                         skip_group_check=True)
                    w2 = sm_p.tile([64 + NN, G], F16, tag="w2", name="w2")
                    nc.scalar.copy(w2[0:NN, :], st["wps"])
                    nc.vector.tensor_copy(w2[64:64 + NN, :], w2[0:NN, :])
                    st["w2"] = w2
                gs.append(whead)

                st["eq"] = []

                def ro_pair(pr):
                    def run():
                        i0, i1 = 2 * pr, 2 * pr + 1
                        if pr == 0:
                            st["embps"] = ps_e.tile([NODE_DIM, G], F32, tag="embps", name="embps")
                        w2 = st["w2"]
                        wtiles = []
                        for i, rlo in ((i0, 0), (i1, 64 if PAIRING else 0)):
                            p, w = CH_OBS[i]
                            wrps = ps_r.tile([128, G], F32, tag="rrps", name="wrps")
                            nc.tensor.matmul(wrps[0:w, :], bl(f"e658t_{i}", rlo, rlo + NN),
                                             w2[rlo:rlo + NN, :], start=True, stop=True,
                                             skip_group_check=True)
                            wtiles.append(wrps)
                        for i, wrps in zip((i0, i1), wtiles):
                            p, w = CH_OBS[i]
                            wnf = wnf_p.tile([128, G], F16, tag="wnf", name="wnf")
                            nc.vector.tensor_mul(wnf[0:w, :], st["obs"][0:w, i, :], wrps[0:w, :])
                            st["eq"].append((i, wnf))
                        while len(st["eq"]) > 2:
                            i, wnf = st["eq"].pop(0)
                            p, w = CH_OBS[i]
                            nc.tensor.matmul(st["embps"], bl(f"kw_{i}", 0, w), wnf[0:w, :],
                                             start=(i == 0), stop=(i == NCH_O - 1),
                                             skip_group_check=True)
                    return run
                for pr in range(NCH_O // 2):
                    gs.append(ro_pair(pr))

                def out():
                    while st["eq"]:
                        i, wnf = st["eq"].pop(0)
                        p, w = CH_OBS[i]
                        nc.tensor.matmul(st["embps"], bl(f"kw_{i}", 0, w), wnf[0:w, :],
                                         start=(i == 0), stop=(i == NCH_O - 1),
                                         skip_group_check=True)
                    emb = out_p.tile([NODE_DIM, G], F16, tag="emb", name="emb")
                    nc.scalar.copy(emb, st["embps"])
                    nc.sync.dma_start(out=embT[:, g0:g0 + G], in_=emb)
                gs.append(out)
                return gs

            def interleave(a, b):
                # proportional round-robin, starting with a
                out, ia, ib, na, nb = [], 0, 0, len(a), len(b)
                while ia < na or ib < nb:
                    if ib < nb and (ia >= na or ib * na <= ia * nb):
                        out.append(b[ib]); ib += 1
                    else:
                        out.append(a[ia]); ia += 1
                return out

            # software pipeline: phase A of block k runs interleaved with phase B
            # of block k-1 so PE/DVE/ACT overlap across blocks.
            sts = {}
            aq = []   # blocks whose A phase is emitted, B pending
            for rep in range(reps):
                for blk in range(N_BLOCKS):
                    key = (rep, blk)
                    sts[key] = {}
                    ga = A_groups(blk, sts[key])
                    if rep == 0 and blk == 0:
                        for fn in ga:
                            fn()
                    elif rep == 0 and blk == 1:
                        # warm-up: overlap A1 with nothing pending yet beyond A0's tail
                        for fn in ga:
                            fn()
                        aq.append((0, (rep, 0)))
                        continue
                    else:
                        bkey = aq.pop(0)
                        gb = B_groups(bkey[0], sts[bkey[1]])
                        for fn in interleave(ga, gb):
                            fn()
                    aq.append((blk, key))
            while aq:
                bblk, bkey = aq.pop(0)
                for fn in B_groups(bblk, sts[bkey]):
                    fn()

    nc.finalize()
    return nc


def _get_program(reps=1):
    key = ("prog", reps, PAIRING, GD_BF16)
    if key not in _CACHE:
        _CACHE[key] = _build_program(reps)
    return _CACHE[key]


# ---------------------------------------------------------------- entry point
def _prep_inputs(observations, W0, b0, Wl, bl, Wg, bg):
    obs = np.asarray(observations, np.float32).reshape(N_GRAPHS, OBS)
    obsT16 = np.zeros((OBS_PAD, N_GRAPHS), np.float16)
    obsT16[:OBS] = obs.T.astype(np.float16)
    obsT16[OBS] = 1.0                                            # virtual ones row

    Wl = np.asarray(Wl, np.float32)
    bl = np.asarray(bl, np.float32)
    Wld16 = (Wl[:, 1::2] - Wl[:, 0::2]).astype(np.float16)       # [64,1225]
    bld = np.zeros((IJ_PAD, 1), np.float32)
    bld[:NN2, 0] = bl[1::2] - bl[0::2]

    import ml_dtypes
    gdT16 = (_gdiff_T() + bld).astype(ml_dtypes.bfloat16 if GD_BF16 else np.float16)

    Wg = np.asarray(Wg, np.float32)
    KW = np.empty((OBS_E, NODE_DIM), np.float32)
    KW[:OBS] = Wg[_KPOS_OF_COL, :]
    KW[OBS] = np.asarray(bg, np.float32)

    blob = _build_blob(np.asarray(W0, np.float32).astype(np.float16),
                       Wld16, KW.astype(np.float16))

    common = {"blob": blob, "b0": np.asarray(b0, np.float32).reshape(HID, 1)}
    in_maps = []
    for c in range(N_CORES):
        s = slice(c * PER_CORE, (c + 1) * PER_CORE)
        m = dict(common)
        m["obsT"] = np.ascontiguousarray(obsT16[:, s])
        m["gdT"] = np.ascontiguousarray(gdT16[:, s])
        in_maps.append(m)
    return in_maps


def _run(inputs, reps=1):
    nc = _get_program(reps)
    in_maps = _prep_inputs(**inputs)
    res = bass_utils.run_bass_kernel_spmd(nc, in_maps, core_ids=list(range(N_CORES)))
    outs = [res.results[c]["embT"] for c in range(N_CORES)]          # each [128, 2048] fp16
    embT = np.concatenate(outs, axis=1).astype(np.float32)           # [128, N]
    return np.ascontiguousarray(embT.T).reshape(T_, B_, NODE_DIM)


def kernel(**inputs):
    return _run(inputs, reps=1)


# revision 42
# speedup vs baseline: 1.0470x; 1.0242x over previous
"""Trainium2 Bass kernel for nn_End2EndGCN (gumbel-softmax GCN over 16384 tiny graphs).

Math (per graph n, derived from the reference):
  obs[658] -> x = relu(W0.T obs + b0)                       [64]
  d(i,j)   = (Wl[:,1]-Wl[:,0]).T x + (bl1-bl0) + gdiff      [35,35]   (softmax over 2 == sigmoid of diff)
  adj      = sigmoid(d)
  deg_i    = sum_j adj_ij ;  r_i = 1/(deg_i + 1e-6)
  w_j      = (1/35) sum_i adj_ij r_i ;  s = sum_j w_j
  emb      = sum_c obs_c * w_node(c) * KW[c,:] + s*bg       [128]     (KW = Wg rows by feature pos;
                                                                       s*bg rides a virtual all-ones obs row)

The gumbel noise gdiff is input-independent (fixed PRNG seed) -> computed once on
host with jax (exact threefry bits, one vmap over all N keys like the reference),
shipped to the device as fp16.

Layout: everything transposed, [feature, graph]; graphs sharded 8 ways (2048/core),
processed in 4 blocks of 512 per core. All matmul operands fp16 (PE runs fp16 at
1 cycle/row vs 4 for fp32), accumulation in fp32 PSUM. GCN normalization is done
with 0/1 selection-matrix matmuls on the PE; the gumbel add is a PE identity-matmul
accumulate. K<=64 matmuls (mm2 / rrep / wrep) are issued in row-group pairs
(tile_position via base partition 0/64) so two run concurrently in the PE array.
All constants ship as one packed [128, C] blob (single DMA); per-block obs/gd each
load with a single 3D-AP DMA from padded host arrays (HWDGE overhead is ~625ns per
dma_start, so DMA count matters).
"""

import numpy as np

import concourse.bass as bass
import concourse.bacc as bacc
import concourse.tile as tile
from concourse import mybir
from concourse import bass_utils

# ---------------------------------------------------------------- problem dims
T_, B_ = 32, 512
N_GRAPHS = T_ * B_            # 16384
OBS = 658
OBS_E = OBS + 1               # +1 virtual all-ones feature row carrying bg*s
HID = 64
NN = 35                       # nodes
NN2 = NN * NN                 # 1225
NODE_DIM = 128
SEED = 1
N_CORES = 8
PER_CORE = N_GRAPHS // N_CORES   # 2048
G = 512                          # graphs per block (PSUM fp32 free-dim limit)
N_BLOCKS = PER_CORE // G         # 4

OBS_PAD = 768                 # padded row counts for single-DMA 3D access patterns
IJ_PAD = 1280

PAIRING = True
GD_BF16 = True
BF16 = mybir.dt.bfloat16
F16 = mybir.dt.float16
F32 = mybir.dt.float32


def _chunks(total, size=128):
    out = []
    p = 0
    while p < total:
        out.append((p, min(size, total - p)))
        p += size
    return out


CH_OBS = _chunks(OBS_E)  # 6 chunks over 659
CH_IJ = _chunks(NN2)     # 10 chunks over 1225


# ---------------------------------------------------------------- host constants
def _feature_map():
    """col -> (node, k) mapping replicating reference.preprocess."""
    node = np.zeros(OBS, np.int64)
    kpos = np.zeros(OBS, np.int64)
    spans = []  # (node, obs_lo, obs_hi, k_lo)
    for i in range(5):                      # hands 5x25
        spans.append((i, 25 * i, 25 * i + 25, 0))
    spans.append((5, 125, 127, 0))          # hands missing card
    spans.append((6, 127, 167, 0))          # deck (40)
    spans.append((7, 167, 192, 0))          # fireworks (25)
    spans.append((8, 192, 200, 0))          # info tokens (8)
    spans.append((9, 200, 203, 0))          # life tokens (3)
    for i in range(5):                      # discards 5x10
        spans.append((10 + i, 203 + 10 * i, 213 + 10 * i, 0))
    la = [(0, 2), (2, 6), (6, 8), (8, 13), (13, 18), (18, 23), (23, 28), (28, 53), (53, 54), (54, 55)]
    for idx, (a, b) in enumerate(la):       # last action slices
        spans.append((15 + idx, 253 + a, 253 + b, 0))
    for i in range(10):                     # v0 nodes: 25 + 5 + 5
        spans.append((25 + i, 308 + 25 * i, 308 + 25 * i + 25, 0))
        spans.append((25 + i, 558 + 5 * i, 558 + 5 * i + 5, 25))
        spans.append((25 + i, 608 + 5 * i, 608 + 5 * i + 5, 30))
    for nd, lo, hi, k0 in spans:
        for c in range(lo, hi):
            node[c] = nd
            kpos[c] = k0 + (c - lo)
    return node, kpos


_NODE_OF_COL, _KPOS_OF_COL = _feature_map()

_CACHE = {}


def _gdiff_T():
    """[IJ_PAD, N] fp32: gumbel(1)-gumbel(0) transposed, zero-padded rows. Input-independent."""
    if "gdT" not in _CACHE:
        import jax

        cpu = jax.devices("cpu")[0]
        with jax.default_device(cpu):
            # Must mirror the reference formulation exactly: one vmap over all
            # N keys (vmap's threefry batching is not chunk-decomposable).
            rng = jax.random.PRNGKey(SEED)
            keys = jax.random.split(rng, N_GRAPHS)
            u = jax.vmap(lambda k: jax.random.uniform(k, (NN, NN, 2)))(keys)
            g = -jax.numpy.log(-jax.numpy.log(u))
            gd = np.asarray(g[..., 1] - g[..., 0], np.float32).reshape(N_GRAPHS, NN2)
        # u==0 gives g=-inf -> gdiff=+/-inf; sigmoid saturates exactly at +/-1000
        # already (matching the reference's softmax limit), and a finite value
        # avoids 0*inf=NaN in the identity-matmul gumbel add.
        gd = np.clip(gd, -1000.0, 1000.0)
        gdT = np.zeros((IJ_PAD, N_GRAPHS), np.float32)
        gdT[:NN2] = gd.T
        _CACHE["gdT"] = gdT
    return _CACHE["gdT"]


class _BlobLayout:
    """Column allocator for the packed [128, C] fp16 constant blob."""

    def __init__(self):
        self.cols = 0
        self.slots = {}

    def alloc(self, name, width):
        self.slots[name] = (self.cols, width)
        self.cols += width
        return self.slots[name]


def _blob_layout():
    L = _BlobLayout()
    for i, (p, w) in enumerate(CH_OBS):
        L.alloc(f"w0_{i}", HID)
    L.alloc("wld2", NN2)
    L.alloc("i128", 128)
    for i, (p, w) in enumerate(CH_IJ):
        L.alloc(f"e35_{i}", NN)
    for i, (p, w) in enumerate(CH_IJ):
        L.alloc(f"e35t_{i}", w)
    for i, (p, w) in enumerate(CH_IJ):
        L.alloc(f"f36_{i}", NN)
    for i, (p, w) in enumerate(CH_OBS):
        L.alloc(f"e658t_{i}", w)
    for i, (p, w) in enumerate(CH_OBS):
        L.alloc(f"kw_{i}", NODE_DIM)
    return L


_LAYOUT = _blob_layout()


def _build_blob(W0, Wld, KW):
    """Host-side packed constant blob [128, C] fp16.

    Selection matrices for the GCN normalization:
      e35_m  [w,35]: deg         degT += e35_m.T @ adjT_m
      e35t_m [35,w]: r broadcast rrepT_m = e35t_m.T @ rT   (dup at rows 64:99 for row-pair packing)
      f36_m  [w,35]: w-sum       wT += f36_m.T @ (adjT*rrepT)_m   (1/35 folded in)
      e658t_i[35,w]: w broadcast over obs cols (+ all-ones virtual row -> s)
      kw_i   [w,128]: fused preprocess-scatter @ Wg (+ bg row)
    """
    ii = np.arange(NN2) // NN
    jj = np.arange(NN2) % NN
    E35 = np.zeros((NN2, NN), np.float16)
    E35[np.arange(NN2), ii] = 1
    F36 = np.zeros((NN2, NN), np.float16)
    F36[np.arange(NN2), jj] = np.float16(1.0 / NN)
    E658T = np.zeros((NN, OBS_E), np.float16)
    E658T[_NODE_OF_COL, np.arange(OBS)] = 1
    E658T[:, OBS] = 1

    blob = np.zeros((128, _LAYOUT.cols), np.float16)

    def put(name, rows, data):
        c0, w = _LAYOUT.slots[name]
        blob[rows, c0:c0 + data.shape[1]] = data

    for i, (p, w) in enumerate(CH_OBS):
        w658 = min(w, OBS - p)
        put(f"w0_{i}", slice(0, w658), W0[p:p + w658, :])
    c0, _ = _LAYOUT.slots["wld2"]
    blob[0:HID, c0:c0 + NN2] = Wld
    blob[64:64 + HID, c0:c0 + NN2] = Wld
    c0, _ = _LAYOUT.slots["i128"]
    blob[:, c0:c0 + 128] = np.eye(128, dtype=np.float16)
    for i, (p, w) in enumerate(CH_IJ):
        put(f"e35_{i}", slice(0, w), E35[p:p + w, :])
        put(f"f36_{i}", slice(0, w), F36[p:p + w, :])
        put(f"e35t_{i}", slice(0, NN), E35[p:p + w, :].T)
        put(f"e35t_{i}", slice(64, 64 + NN), E35[p:p + w, :].T)
    for i, (p, w) in enumerate(CH_OBS):
        put(f"e658t_{i}", slice(0, NN), E658T[:, p:p + w])
        put(f"e658t_{i}", slice(64, 64 + NN), E658T[:, p:p + w])
        put(f"kw_{i}", slice(0, w), KW[p:p + w, :])
    return blob


# ---------------------------------------------------------------- device program
def _build_program(reps=1):
    nc = bacc.Bacc("TRN2", target_bir_lowering=False, debug=False, num_devices=N_CORES)

    obsT = nc.dram_tensor("obsT", [OBS_PAD, PER_CORE], F16, kind="ExternalInput").ap()
    gdT = nc.dram_tensor("gdT", [IJ_PAD, PER_CORE], BF16 if GD_BF16 else F16, kind="ExternalInput").ap()
    blob_d = nc.dram_tensor("blob", [128, _LAYOUT.cols], F16, kind="ExternalInput").ap()
    b0 = nc.dram_tensor("b0", [HID, 1], F32, kind="ExternalInput").ap()
    embT = nc.dram_tensor("embT", [NODE_DIM, PER_CORE], F16, kind="ExternalOutput").ap()

    obsT3 = obsT.rearrange("(m p) g -> p m g", p=128)   # [128, 6, PER_CORE]
    gdT3 = gdT.rearrange("(m p) g -> p m g", p=128)     # [128, 10, PER_CORE]

    AF = mybir.ActivationFunctionType
    NCH_O = len(CH_OBS)
    NCH_IJ = len(CH_IJ)

    with tile.TileContext(nc) as tc:
        with (
            tc.tile_pool(name="singles", bufs=1) as singles,
            tc.tile_pool(name="obs_p", bufs=3) as obs_p,
            tc.tile_pool(name="gd_p", bufs=3) as gd_p,
            tc.tile_pool(name="adj_p", bufs=3) as adj_p,
            tc.tile_pool(name="sm_p", bufs=4) as sm_p,
            tc.tile_pool(name="radj_p", bufs=9) as radj_p,
            tc.tile_pool(name="wnf_p", bufs=9) as wnf_p,
            tc.tile_pool(name="out_p", bufs=3) as out_p,
            tc.tile_pool(name="ps_d", bufs=3, space="PSUM") as ps_d,
            tc.tile_pool(name="ps_deg", bufs=2, space="PSUM") as ps_deg,
            tc.tile_pool(name="ps_r", bufs=2, space="PSUM") as ps_r,
            tc.tile_pool(name="ps_w", bufs=1, space="PSUM") as ps_w,
            tc.tile_pool(name="ps_e", bufs=1, space="PSUM") as ps_e,
        ):
            blob = singles.tile([128, _LAYOUT.cols], F16, tag="blob", name="blob")
            _chead = _LAYOUT.slots["e35_0"][0]   # w0+wld2+i128 prefix needed first
            nc.sync.dma_start(out=blob[:, 0:_chead], in_=blob_d[:, 0:_chead])
            nc.sync.dma_start(out=blob[:, _chead:], in_=blob_d[:, _chead:])
            b0_t = singles.tile([HID, 1], F32, tag="b0", name="b0")
            nc.sync.dma_start(out=b0_t, in_=b0)
            eps_t = singles.tile([NN, 1], F32, tag="eps", name="eps")
            nc.vector.memset(eps_t, 1e-6)

            def bl(name, r0, r1):
                c0, w = _LAYOUT.slots[name]
                return blob[r0:r1, c0:c0 + w]

            cwld, _ = _LAYOUT.slots["wld2"]

            # -------- phase A (adjacency) of block `blk`: list of op-groups
            def A_groups(blk, st):
                g0 = blk * G
                gs = []

                def load():
                    st["obs"] = obs_p.tile([128, NCH_O, G], F16, tag="obs", name="obs_t")
                    nc.sync.dma_start(out=st["obs"], in_=obsT3[:, :, g0:g0 + G])
                    st["gd"] = gd_p.tile([128, NCH_IJ, G], BF16 if GD_BF16 else F16, tag="gd", name="gd_t")
                    nc.sync.dma_start(out=st["gd"], in_=gdT3[:, :, g0:g0 + G])
                gs.append(load)

                def mm1():
                    # x = relu(W0.T obs + b0) [64, G], duplicated at rows 64:128
                    xps = ps_d.tile([128, G], F32, tag="dps", name="xps")
                    for i, (p, w) in enumerate(CH_OBS):
                        w658 = min(w, OBS - p)
                        nc.tensor.matmul(xps[0:HID, :], bl(f"w0_{i}", 0, w658), st["obs"][0:w658, i, :],
                                         start=(i == 0), stop=(i == NCH_O - 1))
                    x2 = sm_p.tile([128, G], F16, tag="x2", name="x2")
                    nc.scalar.activation(x2[0:HID, :], xps[0:HID, :], AF.Relu, bias=b0_t)
                    nc.gpsimd.tensor_copy(x2[64:64 + HID, :], x2[0:HID, :])
                    st["x2"] = x2
                gs.append(mm1)

                st["adj"] = []
                st["degq"] = []

                def adj_pair(pr):
                    def run():
                        m0, m1 = 2 * pr, 2 * pr + 1
                        if pr == 0:
                            st["degps"] = ps_deg.tile([NN, G], F32, tag="degps", name="degps")
                        x2 = st["x2"]
                        dtiles = []
                        for m, rlo in ((m0, 0), (m1, 64 if PAIRING else 0)):
                            p, w = CH_IJ[m]
                            dps = ps_d.tile([128, G], F32, tag="dps", name="dps")
                            nc.tensor.matmul(dps[0:w, :],
                                             blob[rlo:rlo + HID, cwld + p:cwld + p + w],
                                             x2[rlo:rlo + HID, :], start=True, stop=False,
                                             skip_group_check=True)
                            dtiles.append(dps)
                        for m, dps in zip((m0, m1), dtiles):
                            p, w = CH_IJ[m]
                            nc.tensor.matmul(dps[0:w, :], bl("i128", 0, w)[:, 0:w],
                                             st["gd"][0:w, m, :],
                                             start=False, stop=True, skip_group_check=True)
                            adj = adj_p.tile([128, G], F16, tag=f"adj_{m}", name=f"adj_{m}")
                            nc.scalar.activation(adj[0:w, :], dps[0:w, :], AF.Sigmoid)
                            st["adj"].append(adj)
                            st["degq"].append(m)
                        # deg matmuls for the PREVIOUS pair (its sigmoids are done ->
                        # PE does not stall on ACT)
                        while len(st["degq"]) > 4:
                            m = st["degq"].pop(0)
                            p, w = CH_IJ[m]
                            nc.tensor.matmul(st["degps"], bl(f"e35_{m}", 0, w),
                                             st["adj"][m][0:w, :],
                                             start=(m == 0), stop=(m == NCH_IJ - 1),
                                             skip_group_check=True)
                    return run
                for pr in range(NCH_IJ // 2):
                    gs.append(adj_pair(pr))

                def head():
                    while st["degq"]:
                        m = st["degq"].pop(0)
                        p, w = CH_IJ[m]
                        nc.tensor.matmul(st["degps"], bl(f"e35_{m}", 0, w),
                                         st["adj"][m][0:w, :],
                                         start=(m == 0), stop=(m == NCH_IJ - 1),
                                         skip_group_check=True)
                    # r = 1/deg fp16 straight from PSUM, duplicated at rows 64:99.
                    # (reference adds 1e-6; deg >= ~14 for this input distribution,
                    # so the epsilon shifts r by < 1e-7 relative -- far below the
                    # fp16 quantization already present.)
                    r2 = sm_p.tile([64 + NN, G], F16, tag="r2", name="r2")
                    with nc.allow_low_precision("fp16 r is within tolerance"):
                        nc.vector.reciprocal(r2[0:NN, :], st["degps"])
                    # fp16 SBUF copy runs in DVE 4x mode (194ns vs 654ns recip)
                    nc.vector.tensor_copy(r2[64:64 + NN, :], r2[0:NN, :])
                    st["r2"] = r2
                gs.append(head)
                return gs

            # -------- phase B (normalize + readout) of block `blk`
            def B_groups(blk, st):
                g0 = blk * G
                gs = []

                st["wq"] = []

                def rw_pair(pr):
                    def run():
                        m0, m1 = 2 * pr, 2 * pr + 1
                        if pr == 0:
                            st["wps"] = ps_w.tile([NN, G], F32, tag="wps", name="wps")
                        r2 = st["r2"]
                        rtiles = []
                        for m, rlo in ((m0, 0), (m1, 64 if PAIRING else 0)):
                            p, w = CH_IJ[m]
                            rrps = ps_r.tile([128, G], F32, tag="rrps", name="rrps")
                            nc.tensor.matmul(rrps[0:w, :], bl(f"e35t_{m}", rlo, rlo + NN),
                                             r2[rlo:rlo + NN, :], start=True, stop=True,
                                             skip_group_check=True)
                            rtiles.append(rrps)
                        for m, rrps in zip((m0, m1), rtiles):
                            p, w = CH_IJ[m]
                            radj = radj_p.tile([128, G], F16, tag="radj", name="radj")
                            nc.vector.tensor_mul(radj[0:w, :], st["adj"][m][0:w, :], rrps[0:w, :])
                            st["wq"].append((m, radj))
                        while len(st["wq"]) > 6:
                            m, radj = st["wq"].pop(0)
                            p, w = CH_IJ[m]
                            nc.tensor.matmul(st["wps"], bl(f"f36_{m}", 0, w), radj[0:w, :],
                                             start=(m == 0), stop=(m == NCH_IJ - 1),
                                             skip_group_check=True)
                    return run
                for pr in range(NCH_IJ // 2):
                    gs.append(rw_pair(pr))

                def whead():
                    while st["wq"]:
                        m, radj = st["wq"].pop(0)
                        p, w = CH_IJ[m]
                        nc.tensor.matmul(st["wps"], bl(f"f36_{m}", 0, w), radj[0:w, :],
                                         start=(m == 0), stop=(m == NCH_IJ - 1),
                                         skip_group_check=True)
                    w2 = sm_p.tile([64 + NN, G], F16, tag="w2", name="w2")
                    nc.scalar.copy(w2[0:NN, :], st["wps"])
                    nc.vector.tensor_copy(w2[64:64 + NN, :], w2[0:NN, :])
                    st["w2"] = w2
                gs.append(whead)

                st["eq"] = []

                def ro_pair(pr):
                    def run():
                        i0, i1 = 2 * pr, 2 * pr + 1
                        if pr == 0:
                            st["embps"] = ps_e.tile([NODE_DIM, G], F32, tag="embps", name="embps")
                        w2 = st["w2"]
                        wtiles = []
                        for i, rlo in ((i0, 0), (i1, 64 if PAIRING else 0)):
                            p, w = CH_OBS[i]
                            wrps = ps_r.tile([128, G], F32, tag="rrps", name="wrps")
                            nc.tensor.matmul(wrps[0:w, :], bl(f"e658t_{i}", rlo, rlo + NN),
                                             w2[rlo:rlo + NN, :], start=True, stop=True,
                                             skip_group_check=True)
                            wtiles.append(wrps)
                        for i, wrps in zip((i0, i1), wtiles):
                            p, w = CH_OBS[i]
                            wnf = wnf_p.tile([128, G], F16, tag="wnf", name="wnf")
                            nc.vector.tensor_mul(wnf[0:w, :], st["obs"][0:w, i, :], wrps[0:w, :])
                            st["eq"].append((i, wnf))
                        while len(st["eq"]) > 6:
                            i, wnf = st["eq"].pop(0)
                            p, w = CH_OBS[i]
                            nc.tensor.matmul(st["embps"], bl(f"kw_{i}", 0, w), wnf[0:w, :],
                                             start=(i == 0), stop=(i == NCH_O - 1),
                                             skip_group_check=True)
                    return run
                for pr in range(NCH_O // 2):
                    gs.append(ro_pair(pr))

                def out():
                    while st["eq"]:
                        i, wnf = st["eq"].pop(0)
                        p, w = CH_OBS[i]
                        nc.tensor.matmul(st["embps"], bl(f"kw_{i}", 0, w), wnf[0:w, :],
                                         start=(i == 0), stop=(i == NCH_O - 1),
                                         skip_group_check=True)
                    emb = out_p.tile([NODE_DIM, G], F16, tag="emb", name="emb")
                    nc.scalar.copy(emb, st["embps"])
                    nc.sync.dma_start(out=embT[:, g0:g0 + G], in_=emb)
                gs.append(out)
                return gs

            def interleave(a, b):
                # proportional round-robin, starting with a
                out, ia, ib, na, nb = [], 0, 0, len(a), len(b)
                while ia < na or ib < nb:
                    if ib < nb and (ia >= na or ib * na <= ia * nb):
                        out.append(b[ib]); ib += 1
                    else:
                        out.append(a[ia]); ia += 1
                return out

            # software pipeline: phase A of block k runs interleaved with phase B
            # of block k-1 so PE/DVE/ACT overlap across blocks.
            sts = {}
            aq = []   # blocks whose A phase is emitted, B pending
            for rep in range(reps):
                for blk in range(N_BLOCKS):
                    key = (rep, blk)
                    sts[key] = {}
                    ga = A_groups(blk, sts[key])
                    if rep == 0 and blk == 0:
                        for fn in ga:
                            fn()
                    elif rep == 0 and blk == 1:
                        # warm-up: overlap A1 with nothing pending yet beyond A0's tail
                        for fn in ga:
                            fn()
                        aq.append((0, (rep, 0)))
                        continue
                    else:
                        bkey = aq.pop(0)
                        gb = B_groups(bkey[0], sts[bkey[1]])
                        for fn in interleave(ga, gb):
                            fn()
                    aq.append((blk, key))
            while aq:
                bblk, bkey = aq.pop(0)
                for fn in B_groups(bblk, sts[bkey]):
                    fn()

    nc.finalize()
    return nc


def _get_program(reps=1):
    key = ("prog", reps, PAIRING, GD_BF16)
    if key not in _CACHE:
        _CACHE[key] = _build_program(reps)
    return _CACHE[key]


# ---------------------------------------------------------------- entry point
def _prep_inputs(observations, W0, b0, Wl, bl, Wg, bg):
    obs = np.asarray(observations, np.float32).reshape(N_GRAPHS, OBS)
    obsT16 = np.zeros((OBS_PAD, N_GRAPHS), np.float16)
    obsT16[:OBS] = obs.T.astype(np.float16)
    obsT16[OBS] = 1.0                                            # virtual ones row

    Wl = np.asarray(Wl, np.float32)
    bl = np.asarray(bl, np.float32)
    Wld16 = (Wl[:, 1::2] - Wl[:, 0::2]).astype(np.float16)       # [64,1225]
    bld = np.zeros((IJ_PAD, 1), np.float32)
    bld[:NN2, 0] = bl[1::2] - bl[0::2]

    import ml_dtypes
    gdT16 = (_gdiff_T() + bld).astype(ml_dtypes.bfloat16 if GD_BF16 else np.float16)

    Wg = np.asarray(Wg, np.float32)
    KW = np.empty((OBS_E, NODE_DIM), np.float32)
    KW[:OBS] = Wg[_KPOS_OF_COL, :]
    KW[OBS] = np.asarray(bg, np.float32)

    blob = _build_blob(np.asarray(W0, np.float32).astype(np.float16),
                       Wld16, KW.astype(np.float16))

    common = {"blob": blob, "b0": np.asarray(b0, np.float32).reshape(HID, 1)}
    in_maps = []
    for c in range(N_CORES):
        s = slice(c * PER_CORE, (c + 1) * PER_CORE)
        m = dict(common)
        m["obsT"] = np.ascontiguousarray(obsT16[:, s])
        m["gdT"] = np.ascontiguousarray(gdT16[:, s])
        in_maps.append(m)
    return in_maps


def _run(inputs, reps=1):
    nc = _get_program(reps)
    in_maps = _prep_inputs(**inputs)
    res = bass_utils.run_bass_kernel_spmd(nc, in_maps, core_ids=list(range(N_CORES)))
    outs = [res.results[c]["embT"] for c in range(N_CORES)]          # each [128, 2048] fp16
    embT = np.concatenate(outs, axis=1).astype(np.float32)           # [128, N]
    return np.ascontiguousarray(embT.T).reshape(T_, B_, NODE_DIM)


def kernel(**inputs):
    return _run(inputs, reps=1)


# revision 43
# speedup vs baseline: 1.0525x; 1.0053x over previous
"""Trainium2 Bass kernel for nn_End2EndGCN (gumbel-softmax GCN over 16384 tiny graphs).

Math (per graph n, derived from the reference):
  obs[658] -> x = relu(W0.T obs + b0)                       [64]
  d(i,j)   = (Wl[:,1]-Wl[:,0]).T x + (bl1-bl0) + gdiff      [35,35]   (softmax over 2 == sigmoid of diff)
  adj      = sigmoid(d)
  deg_i    = sum_j adj_ij ;  r_i = 1/(deg_i + 1e-6)
  w_j      = (1/35) sum_i adj_ij r_i ;  s = sum_j w_j
  emb      = sum_c obs_c * w_node(c) * KW[c,:] + s*bg       [128]     (KW = Wg rows by feature pos;
                                                                       s*bg rides a virtual all-ones obs row)

The gumbel noise gdiff is input-independent (fixed PRNG seed) -> computed once on
host with jax (exact threefry bits, one vmap over all N keys like the reference),
shipped to the device as fp16.

Layout: everything transposed, [feature, graph]; graphs sharded 8 ways (2048/core),
processed in 4 blocks of 512 per core. All matmul operands fp16 (PE runs fp16 at
1 cycle/row vs 4 for fp32), accumulation in fp32 PSUM. GCN normalization is done
with 0/1 selection-matrix matmuls on the PE; the gumbel add is a PE identity-matmul
accumulate. K<=64 matmuls (mm2 / rrep / wrep) are issued in row-group pairs
(tile_position via base partition 0/64) so two run concurrently in the PE array.
All constants ship as one packed [128, C] blob (single DMA); per-block obs/gd each
load with a single 3D-AP DMA from padded host arrays (HWDGE overhead is ~625ns per
dma_start, so DMA count matters).
"""

import numpy as np

import concourse.bass as bass
import concourse.bacc as bacc
import concourse.tile as tile
from concourse import mybir
from concourse import bass_utils

# ---------------------------------------------------------------- problem dims
T_, B_ = 32, 512
N_GRAPHS = T_ * B_            # 16384
OBS = 658
OBS_E = OBS + 1               # +1 virtual all-ones feature row carrying bg*s
HID = 64
NN = 35                       # nodes
NN2 = NN * NN                 # 1225
NODE_DIM = 128
SEED = 1
N_CORES = 8
PER_CORE = N_GRAPHS // N_CORES   # 2048
G = 512                          # graphs per block (PSUM fp32 free-dim limit)
N_BLOCKS = PER_CORE // G         # 4

OBS_PAD = 768                 # padded row counts for single-DMA 3D access patterns
IJ_PAD = 1280

PAIRING = True
GD_BF16 = True
BF16 = mybir.dt.bfloat16
F16 = mybir.dt.float16
F32 = mybir.dt.float32


def _chunks(total, size=128):
    out = []
    p = 0
    while p < total:
        out.append((p, min(size, total - p)))
        p += size
    return out


CH_OBS = _chunks(OBS_E)  # 6 chunks over 659
CH_IJ = _chunks(NN2)     # 10 chunks over 1225


# ---------------------------------------------------------------- host constants
def _feature_map():
    """col -> (node, k) mapping replicating reference.preprocess."""
    node = np.zeros(OBS, np.int64)
    kpos = np.zeros(OBS, np.int64)
    spans = []  # (node, obs_lo, obs_hi, k_lo)
    for i in range(5):                      # hands 5x25
        spans.append((i, 25 * i, 25 * i + 25, 0))
    spans.append((5, 125, 127, 0))          # hands missing card
    spans.append((6, 127, 167, 0))          # deck (40)
    spans.append((7, 167, 192, 0))          # fireworks (25)
    spans.append((8, 192, 200, 0))          # info tokens (8)
    spans.append((9, 200, 203, 0))          # life tokens (3)
    for i in range(5):                      # discards 5x10
        spans.append((10 + i, 203 + 10 * i, 213 + 10 * i, 0))
    la = [(0, 2), (2, 6), (6, 8), (8, 13), (13, 18), (18, 23), (23, 28), (28, 53), (53, 54), (54, 55)]
    for idx, (a, b) in enumerate(la):       # last action slices
        spans.append((15 + idx, 253 + a, 253 + b, 0))
    for i in range(10):                     # v0 nodes: 25 + 5 + 5
        spans.append((25 + i, 308 + 25 * i, 308 + 25 * i + 25, 0))
        spans.append((25 + i, 558 + 5 * i, 558 + 5 * i + 5, 25))
        spans.append((25 + i, 608 + 5 * i, 608 + 5 * i + 5, 30))
    for nd, lo, hi, k0 in spans:
        for c in range(lo, hi):
            node[c] = nd
            kpos[c] = k0 + (c - lo)
    return node, kpos


_NODE_OF_COL, _KPOS_OF_COL = _feature_map()

_CACHE = {}


def _gdiff_T():
    """[IJ_PAD, N] fp32: gumbel(1)-gumbel(0) transposed, zero-padded rows. Input-independent."""
    if "gdT" not in _CACHE:
        import jax

        cpu = jax.devices("cpu")[0]
        with jax.default_device(cpu):
            # Must mirror the reference formulation exactly: one vmap over all
            # N keys (vmap's threefry batching is not chunk-decomposable).
            rng = jax.random.PRNGKey(SEED)
            keys = jax.random.split(rng, N_GRAPHS)
            u = jax.vmap(lambda k: jax.random.uniform(k, (NN, NN, 2)))(keys)
            g = -jax.numpy.log(-jax.numpy.log(u))
            gd = np.asarray(g[..., 1] - g[..., 0], np.float32).reshape(N_GRAPHS, NN2)
        # u==0 gives g=-inf -> gdiff=+/-inf; sigmoid saturates exactly at +/-1000
        # already (matching the reference's softmax limit), and a finite value
        # avoids 0*inf=NaN in the identity-matmul gumbel add.
        gd = np.clip(gd, -1000.0, 1000.0)
        gdT = np.zeros((IJ_PAD, N_GRAPHS), np.float32)
        gdT[:NN2] = gd.T
        _CACHE["gdT"] = gdT
    return _CACHE["gdT"]


class _BlobLayout:
    """Column allocator for the packed [128, C] fp16 constant blob."""

    def __init__(self):
        self.cols = 0
        self.slots = {}

    def alloc(self, name, width):
        self.slots[name] = (self.cols, width)
        self.cols += width
        return self.slots[name]


def _blob_layout():
    L = _BlobLayout()
    for i, (p, w) in enumerate(CH_OBS):
        L.alloc(f"w0_{i}", HID)
    L.alloc("wld2", NN2)
    L.alloc("i128", 128)
    for i, (p, w) in enumerate(CH_IJ):
        L.alloc(f"e35_{i}", NN)
    for i, (p, w) in enumerate(CH_IJ):
        L.alloc(f"e35t_{i}", w)
    for i, (p, w) in enumerate(CH_IJ):
        L.alloc(f"f36_{i}", NN)
    for i, (p, w) in enumerate(CH_OBS):
        L.alloc(f"e658t_{i}", w)
    for i, (p, w) in enumerate(CH_OBS):
        L.alloc(f"kw_{i}", NODE_DIM)
    return L


_LAYOUT = _blob_layout()


def _build_blob(W0, Wld, KW):
    """Host-side packed constant blob [128, C] fp16.

    Selection matrices for the GCN normalization:
      e35_m  [w,35]: deg         degT += e35_m.T @ adjT_m
      e35t_m [35,w]: r broadcast rrepT_m = e35t_m.T @ rT   (dup at rows 64:99 for row-pair packing)
      f36_m  [w,35]: w-sum       wT += f36_m.T @ (adjT*rrepT)_m   (1/35 folded in)
      e658t_i[35,w]: w broadcast over obs cols (+ all-ones virtual row -> s)
      kw_i   [w,128]: fused preprocess-scatter @ Wg (+ bg row)
    """
    ii = np.arange(NN2) // NN
    jj = np.arange(NN2) % NN
    E35 = np.zeros((NN2, NN), np.float16)
    E35[np.arange(NN2), ii] = 1
    F36 = np.zeros((NN2, NN), np.float16)
    F36[np.arange(NN2), jj] = np.float16(1.0 / NN)
    E658T = np.zeros((NN, OBS_E), np.float16)
    E658T[_NODE_OF_COL, np.arange(OBS)] = 1
    E658T[:, OBS] = 1

    blob = np.zeros((128, _LAYOUT.cols), np.float16)

    def put(name, rows, data):
        c0, w = _LAYOUT.slots[name]
        blob[rows, c0:c0 + data.shape[1]] = data

    for i, (p, w) in enumerate(CH_OBS):
        w658 = min(w, OBS - p)
        put(f"w0_{i}", slice(0, w658), W0[p:p + w658, :])
    c0, _ = _LAYOUT.slots["wld2"]
    blob[0:HID, c0:c0 + NN2] = Wld
    blob[64:64 + HID, c0:c0 + NN2] = Wld
    c0, _ = _LAYOUT.slots["i128"]
    blob[:, c0:c0 + 128] = np.eye(128, dtype=np.float16)
    for i, (p, w) in enumerate(CH_IJ):
        put(f"e35_{i}", slice(0, w), E35[p:p + w, :])
        put(f"f36_{i}", slice(0, w), F36[p:p + w, :])
        put(f"e35t_{i}", slice(0, NN), E35[p:p + w, :].T)
        put(f"e35t_{i}", slice(64, 64 + NN), E35[p:p + w, :].T)
    for i, (p, w) in enumerate(CH_OBS):
        put(f"e658t_{i}", slice(0, NN), E658T[:, p:p + w])
        put(f"e658t_{i}", slice(64, 64 + NN), E658T[:, p:p + w])
        put(f"kw_{i}", slice(0, w), KW[p:p + w, :])
    return blob


# ---------------------------------------------------------------- device program
def _build_program(reps=1):
    nc = bacc.Bacc("TRN2", target_bir_lowering=False, debug=False, num_devices=N_CORES)

    obsT = nc.dram_tensor("obsT", [OBS_PAD, PER_CORE], F16, kind="ExternalInput").ap()
    gdT = nc.dram_tensor("gdT", [IJ_PAD, PER_CORE], BF16 if GD_BF16 else F16, kind="ExternalInput").ap()
    blob_d = nc.dram_tensor("blob", [128, _LAYOUT.cols], F16, kind="ExternalInput").ap()
    b0 = nc.dram_tensor("b0", [HID, 1], F32, kind="ExternalInput").ap()
    embT = nc.dram_tensor("embT", [NODE_DIM, PER_CORE], F16, kind="ExternalOutput").ap()

    obsT3 = obsT.rearrange("(m p) g -> p m g", p=128)   # [128, 6, PER_CORE]
    gdT3 = gdT.rearrange("(m p) g -> p m g", p=128)     # [128, 10, PER_CORE]

    AF = mybir.ActivationFunctionType
    NCH_O = len(CH_OBS)
    NCH_IJ = len(CH_IJ)

    with tile.TileContext(nc) as tc:
        with (
            tc.tile_pool(name="singles", bufs=1) as singles,
            tc.tile_pool(name="obs_p", bufs=4) as obs_p,
            tc.tile_pool(name="gd_p", bufs=4) as gd_p,
            tc.tile_pool(name="adj_p", bufs=4) as adj_p,
            tc.tile_pool(name="sm_p", bufs=5) as sm_p,
            tc.tile_pool(name="radj_p", bufs=9) as radj_p,
            tc.tile_pool(name="wnf_p", bufs=9) as wnf_p,
            tc.tile_pool(name="out_p", bufs=3) as out_p,
            tc.tile_pool(name="ps_d", bufs=3, space="PSUM") as ps_d,
            tc.tile_pool(name="ps_deg", bufs=2, space="PSUM") as ps_deg,
            tc.tile_pool(name="ps_r", bufs=2, space="PSUM") as ps_r,
            tc.tile_pool(name="ps_w", bufs=1, space="PSUM") as ps_w,
            tc.tile_pool(name="ps_e", bufs=1, space="PSUM") as ps_e,
        ):
            blob = singles.tile([128, _LAYOUT.cols], F16, tag="blob", name="blob")
            _chead = _LAYOUT.slots["e35_0"][0]   # w0+wld2+i128 prefix needed first
            nc.sync.dma_start(out=blob[:, 0:_chead], in_=blob_d[:, 0:_chead])
            nc.sync.dma_start(out=blob[:, _chead:], in_=blob_d[:, _chead:])
            b0_t = singles.tile([HID, 1], F32, tag="b0", name="b0")
            nc.sync.dma_start(out=b0_t, in_=b0)
            eps_t = singles.tile([NN, 1], F32, tag="eps", name="eps")
            nc.vector.memset(eps_t, 1e-6)

            def bl(name, r0, r1):
                c0, w = _LAYOUT.slots[name]
                return blob[r0:r1, c0:c0 + w]

            cwld, _ = _LAYOUT.slots["wld2"]

            # -------- phase A (adjacency) of block `blk`: list of op-groups
            def A_groups(blk, st):
                g0 = blk * G
                gs = []

                def load():
                    st["obs"] = obs_p.tile([128, NCH_O, G], F16, tag="obs", name="obs_t")
                    nc.sync.dma_start(out=st["obs"], in_=obsT3[:, :, g0:g0 + G])
                    st["gd"] = gd_p.tile([128, NCH_IJ, G], BF16 if GD_BF16 else F16, tag="gd", name="gd_t")
                    nc.sync.dma_start(out=st["gd"], in_=gdT3[:, :, g0:g0 + G])
                gs.append(load)

                def mm1():
                    # x = relu(W0.T obs + b0) [64, G], duplicated at rows 64:128
                    xps = ps_d.tile([128, G], F32, tag="dps", name="xps")
                    for i, (p, w) in enumerate(CH_OBS):
                        w658 = min(w, OBS - p)
                        nc.tensor.matmul(xps[0:HID, :], bl(f"w0_{i}", 0, w658), st["obs"][0:w658, i, :],
                                         start=(i == 0), stop=(i == NCH_O - 1))
                    x2 = sm_p.tile([128, G], F16, tag="x2", name="x2")
                    nc.scalar.activation(x2[0:HID, :], xps[0:HID, :], AF.Relu, bias=b0_t)
                    nc.gpsimd.tensor_copy(x2[64:64 + HID, :], x2[0:HID, :])
                    st["x2"] = x2
                gs.append(mm1)

                st["adj"] = []
                st["degq"] = []

                def adj_pair(pr):
                    def run():
                        m0, m1 = 2 * pr, 2 * pr + 1
                        if pr == 0:
                            st["degps"] = ps_deg.tile([NN, G], F32, tag="degps", name="degps")
                        x2 = st["x2"]
                        dtiles = []
                        for m, rlo in ((m0, 0), (m1, 64 if PAIRING else 0)):
                            p, w = CH_IJ[m]
                            dps = ps_d.tile([128, G], F32, tag="dps", name="dps")
                            nc.tensor.matmul(dps[0:w, :],
                                             blob[rlo:rlo + HID, cwld + p:cwld + p + w],
                                             x2[rlo:rlo + HID, :], start=True, stop=False,
                                             skip_group_check=True)
                            dtiles.append(dps)
                        for m, dps in zip((m0, m1), dtiles):
                            p, w = CH_IJ[m]
                            nc.tensor.matmul(dps[0:w, :], bl("i128", 0, w)[:, 0:w],
                                             st["gd"][0:w, m, :],
                                             start=False, stop=True, skip_group_check=True)
                            adj = adj_p.tile([128, G], F16, tag=f"adj_{m}", name=f"adj_{m}")
                            nc.scalar.activation(adj[0:w, :], dps[0:w, :], AF.Sigmoid)
                            st["adj"].append(adj)
                            st["degq"].append(m)
                        # deg matmuls for the PREVIOUS pair (its sigmoids are done ->
                        # PE does not stall on ACT)
                        while len(st["degq"]) > 6:
                            m = st["degq"].pop(0)
                            p, w = CH_IJ[m]
                            nc.tensor.matmul(st["degps"], bl(f"e35_{m}", 0, w),
                                             st["adj"][m][0:w, :],
                                             start=(m == 0), stop=(m == NCH_IJ - 1),
                                             skip_group_check=True)
                    return run
                for pr in range(NCH_IJ // 2):
                    gs.append(adj_pair(pr))

                def head():
                    while st["degq"]:
                        m = st["degq"].pop(0)
                        p, w = CH_IJ[m]
                        nc.tensor.matmul(st["degps"], bl(f"e35_{m}", 0, w),
                                         st["adj"][m][0:w, :],
                                         start=(m == 0), stop=(m == NCH_IJ - 1),
                                         skip_group_check=True)
                    # r = 1/deg fp16 straight from PSUM, duplicated at rows 64:99.
                    # (reference adds 1e-6; deg >= ~14 for this input distribution,
                    # so the epsilon shifts r by < 1e-7 relative -- far below the
                    # fp16 quantization already present.)
                    r2 = sm_p.tile([64 + NN, G], F16, tag="r2", name="r2")
                    with nc.allow_low_precision("fp16 r is within tolerance"):
                        nc.vector.reciprocal(r2[0:NN, :], st["degps"])
                    # fp16 SBUF copy runs in DVE 4x mode (194ns vs 654ns recip)
                    nc.vector.tensor_copy(r2[64:64 + NN, :], r2[0:NN, :])
                    st["r2"] = r2
                gs.append(head)
                return gs

            # -------- phase B (normalize + readout) of block `blk`
            def B_groups(blk, st):
                g0 = blk * G
                gs = []

                st["wq"] = []

                def rw_pair(pr):
                    def run():
                        m0, m1 = 2 * pr, 2 * pr + 1
                        if pr == 0:
                            st["wps"] = ps_w.tile([NN, G], F32, tag="wps", name="wps")
                        r2 = st["r2"]
                        rtiles = []
                        for m, rlo in ((m0, 0), (m1, 64 if PAIRING else 0)):
                            p, w = CH_IJ[m]
                            rrps = ps_r.tile([128, G], F32, tag="rrps", name="rrps")
                            nc.tensor.matmul(rrps[0:w, :], bl(f"e35t_{m}", rlo, rlo + NN),
                                             r2[rlo:rlo + NN, :], start=True, stop=True,
                                             skip_group_check=True)
                            rtiles.append(rrps)
                        for m, rrps in zip((m0, m1), rtiles):
                            p, w = CH_IJ[m]
                            radj = radj_p.tile([128, G], F16, tag="radj", name="radj")
                            nc.vector.tensor_mul(radj[0:w, :], st["adj"][m][0:w, :], rrps[0:w, :])
                            st["wq"].append((m, radj))
                        while len(st["wq"]) > 6:
                            m, radj = st["wq"].pop(0)
                            p, w = CH_IJ[m]
                            nc.tensor.matmul(st["wps"], bl(f"f36_{m}", 0, w), radj[0:w, :],
                                             start=(m == 0), stop=(m == NCH_IJ - 1),
                                             skip_group_check=True)
                    return run
                for pr in range(NCH_IJ // 2):
                    gs.append(rw_pair(pr))

                def whead():
                    while st["wq"]:
                        m, radj = st["wq"].pop(0)
                        p, w = CH_IJ[m]
                        nc.tensor.matmul(st["wps"], bl(f"f36_{m}", 0, w), radj[0:w, :],
                                         start=(m == 0), stop=(m == NCH_IJ - 1),
                                         skip_group_check=True)
                    w2 = sm_p.tile([64 + NN, G], F16, tag="w2", name="w2")
                    nc.scalar.copy(w2[0:NN, :], st["wps"])
                    nc.vector.tensor_copy(w2[64:64 + NN, :], w2[0:NN, :])
                    st["w2"] = w2
                gs.append(whead)

                st["eq"] = []

                def ro_pair(pr):
                    def run():
                        i0, i1 = 2 * pr, 2 * pr + 1
                        if pr == 0:
                            st["embps"] = ps_e.tile([NODE_DIM, G], F32, tag="embps", name="embps")
                        w2 = st["w2"]
                        wtiles = []
                        for i, rlo in ((i0, 0), (i1, 64 if PAIRING else 0)):
                            p, w = CH_OBS[i]
                            wrps = ps_r.tile([128, G], F32, tag="rrps", name="wrps")
                            nc.tensor.matmul(wrps[0:w, :], bl(f"e658t_{i}", rlo, rlo + NN),
                                             w2[rlo:rlo + NN, :], start=True, stop=True,
                                             skip_group_check=True)
                            wtiles.append(wrps)
                        for i, wrps in zip((i0, i1), wtiles):
                            p, w = CH_OBS[i]
                            wnf = wnf_p.tile([128, G], F16, tag="wnf", name="wnf")
                            nc.vector.tensor_mul(wnf[0:w, :], st["obs"][0:w, i, :], wrps[0:w, :])
                            st["eq"].append((i, wnf))
                        while len(st["eq"]) > 6:
                            i, wnf = st["eq"].pop(0)
                            p, w = CH_OBS[i]
                            nc.tensor.matmul(st["embps"], bl(f"kw_{i}", 0, w), wnf[0:w, :],
                                             start=(i == 0), stop=(i == NCH_O - 1),
                                             skip_group_check=True)
                    return run
                for pr in range(NCH_O // 2):
                    gs.append(ro_pair(pr))

                def out():
                    while st["eq"]:
                        i, wnf = st["eq"].pop(0)
                        p, w = CH_OBS[i]
                        nc.tensor.matmul(st["embps"], bl(f"kw_{i}", 0, w), wnf[0:w, :],
                                         start=(i == 0), stop=(i == NCH_O - 1),
                                         skip_group_check=True)
                    emb = out_p.tile([NODE_DIM, G], F16, tag="emb", name="emb")
                    nc.scalar.copy(emb, st["embps"])
                    nc.sync.dma_start(out=embT[:, g0:g0 + G], in_=emb)
                gs.append(out)
                return gs

            def interleave(a, b):
                # proportional round-robin, starting with a
                out, ia, ib, na, nb = [], 0, 0, len(a), len(b)
                while ia < na or ib < nb:
                    if ib < nb and (ia >= na or ib * na <= ia * nb):
                        out.append(b[ib]); ib += 1
                    else:
                        out.append(a[ia]); ia += 1
                return out

            # software pipeline: phase A of block k runs interleaved with phase B
            # of block k-1 so PE/DVE/ACT overlap across blocks.
            sts = {}
            aq = []   # blocks whose A phase is emitted, B pending
            for rep in range(reps):
                for blk in range(N_BLOCKS):
                    key = (rep, blk)
                    sts[key] = {}
                    ga = A_groups(blk, sts[key])
                    if rep == 0 and blk == 0:
                        for fn in ga:
                            fn()
                    elif rep == 0 and blk == 1:
                        # warm-up: overlap A1 with nothing pending yet beyond A0's tail
                        for fn in ga:
                            fn()
                        aq.append((0, (rep, 0)))
                        continue
                    else:
                        bkey = aq.pop(0)
                        gb = B_groups(bkey[0], sts[bkey[1]])
                        for fn in interleave(ga, gb):
                            fn()
                    aq.append((blk, key))
            while aq:
                bblk, bkey = aq.pop(0)
                for fn in B_groups(bblk, sts[bkey]):
                    fn()

    nc.finalize()
    return nc


def _get_program(reps=1):
    key = ("prog", reps, PAIRING, GD_BF16)
    if key not in _CACHE:
        _CACHE[key] = _build_program(reps)
    return _CACHE[key]


# ---------------------------------------------------------------- entry point
def _prep_inputs(observations, W0, b0, Wl, bl, Wg, bg):
    obs = np.asarray(observations, np.float32).reshape(N_GRAPHS, OBS)
    obsT16 = np.zeros((OBS_PAD, N_GRAPHS), np.float16)
    obsT16[:OBS] = obs.T.astype(np.float16)
    obsT16[OBS] = 1.0                                            # virtual ones row

    Wl = np.asarray(Wl, np.float32)
    bl = np.asarray(bl, np.float32)
    Wld16 = (Wl[:, 1::2] - Wl[:, 0::2]).astype(np.float16)       # [64,1225]
    bld = np.zeros((IJ_PAD, 1), np.float32)
    bld[:NN2, 0] = bl[1::2] - bl[0::2]

    import ml_dtypes
    gdT16 = (_gdiff_T() + bld).astype(ml_dtypes.bfloat16 if GD_BF16 else np.float16)

    Wg = np.asarray(Wg, np.float32)
    KW = np.empty((OBS_E, NODE_DIM), np.float32)
    KW[:OBS] = Wg[_KPOS_OF_COL, :]
    KW[OBS] = np.asarray(bg, np.float32)

    blob = _build_blob(np.asarray(W0, np.float32).astype(np.float16),
                       Wld16, KW.astype(np.float16))

    common = {"blob": blob, "b0": np.asarray(b0, np.float32).reshape(HID, 1)}
    in_maps = []
    for c in range(N_CORES):
        s = slice(c * PER_CORE, (c + 1) * PER_CORE)
        m = dict(common)
        m["obsT"] = np.ascontiguousarray(obsT16[:, s])
        m["gdT"] = np.ascontiguousarray(gdT16[:, s])
        in_maps.append(m)
    return in_maps


def _run(inputs, reps=1):
    nc = _get_program(reps)
    in_maps = _prep_inputs(**inputs)
    res = bass_utils.run_bass_kernel_spmd(nc, in_maps, core_ids=list(range(N_CORES)))
    outs = [res.results[c]["embT"] for c in range(N_CORES)]          # each [128, 2048] fp16
    embT = np.concatenate(outs, axis=1).astype(np.float32)           # [128, N]
    return np.ascontiguousarray(embT.T).reshape(T_, B_, NODE_DIM)


def kernel(**inputs):
    return _run(inputs, reps=1)
